# revision 40
# baseline (speedup 1.0000x reference)
"""Multi-Head Latent Attention (MLA) forward pass on 8 Trainium2 NeuronCores.

Sharding: num_heads tensor-parallel (2 heads/core) for up-projections,
attention and out-proj; the low-rank down-projections + LayerNorm are
token-parallel (512 tokens/core) followed by on-device AllGathers of the
bf16 latents (kv first, overlapped with the q path). Per-core partial
outputs (out-proj with input-dim-sliced Wout, bf16) are summed on the host.

Schedule: a tiny warmup collective absorbs the CC barrier/cold-start;
the kv AllGather is triggered as soon as the kv latents are normalized
(its LN runs while the q down-proj matmuls keep PE busy); the q AllGather
rides under the kv up-projection path.  Attention is software-pipelined:
scores for key-tile s+3 are issued before the PV matmuls of tile s, the
exp/row-sum run batched over both local heads ([128,1024] tiles), the
softmax denominator is reduced on the (otherwise idle) GpSimd engine,
and the out-projection of block b-1 is interleaved into block b's score
stream so the PE never drains.

Self-contained: hardcodes all shapes from the problem spec.
"""

from contextlib import ExitStack

import numpy as np
import ml_dtypes

import concourse.bass as bass
import concourse.mybir as mybir
import concourse.tile as tile
from concourse import bacc
from concourse.bass_isa import ReduceOp
from concourse.bass_utils import run_bass_kernel_spmd
from concourse.masks import make_identity

# ---- problem dimensions (hardcoded) ----
NCORES = 8
P = 128
B = 2
S = 2048           # sequence length
T = B * S          # total tokens = 4096
D = 2048           # d_model
QR = 1536          # q rank
KVR = 512          # kv rank
H = 16             # heads
HD = 128           # head dim (content)
RD = 64            # rope dim
HLOC = H // NCORES # heads per core = 2
TLOC = T // NCORES # tokens per core = 512
NQ = HLOC * HD     # 256 per-core content out dims
NR = HLOC * RD     # 128 per-core rope out dims
SCALE = (HD + RD) ** -0.5
LN_EPS = 1e-5

BF = mybir.dt.bfloat16
F32 = mybir.dt.float32
AX = mybir.AxisListType
OP = mybir.AluOpType
ACT = mybir.ActivationFunctionType

NKT = S // P       # 16 key tiles per sequence
KQ = QR // P       # 12
KKV = KVR // P     # 4
KX = D // P        # 16
MT = TLOC // P     # 4 token tiles per core
NBLK = B * (S // 512)  # 8 attention blocks of 512 q tokens
AV_LAG = 3         # PV matmuls trail the score matmuls by this many tiles


def build(has_bias: bool, phases: int = 3):
    nc = bacc.Bacc("TRN2", target_bir_lowering=False, debug=False,
                   num_devices=NCORES, enable_asserts=False)

    def din(name, shape, dt=BF):
        return nc.dram_tensor(name, shape, dt, kind="ExternalInput").ap()

    xt = din("xt", [D, TLOC])
    wq_down = din("wq_down", [D, QR])
    wkv_down = din("wkv_down", [D, KVR])
    gq_up = din("gq_up", [QR, NQ])
    gq_rope = din("gq_rope", [QR, NR])
    gq_rope_rot = din("gq_rope_rot", [QR, NR])
    gk_up = din("gk_up", [KVR, NQ])
    gk_rope = din("gk_rope", [KVR, NR])
    gk_rope_rot = din("gk_rope_rot", [KVR, NR])
    gv_up = din("gv_up", [KVR, NQ])
    wout = din("wout", [NQ, D])
    cos_t = din("cos_t", [NR, T])
    sin_t = din("sin_t", [NR, T])
    if has_bias:
        bq_up = din("bq_up", [1, NQ])
        bq_rope = din("bq_rope", [1, NR])
        bq_rope_rot = din("bq_rope_rot", [1, NR])
        bk_up = din("bk_up", [1, NQ])
        bk_rope = din("bk_rope", [1, NR])
        bk_rope_rot = din("bk_rope_rot", [1, NR])
        bv_up = din("bv_up", [1, NQ])
    out_p = nc.dram_tensor("out_p", [T, D], BF, kind="ExternalOutput").ap()

    agi_kv = nc.dram_tensor("agi_kv", [KVR, TLOC], BF).ap()
    ago_kv = nc.dram_tensor("ago_kv", [NCORES * KVR, TLOC], BF,
                            addr_space="Shared").ap()
    agi_qh = [nc.dram_tensor(f"agi_q{i}", [QR, TLOC // 2], BF).ap()
              for i in range(2)]
    ago_qh = [nc.dram_tensor(f"ago_q{i}", [NCORES * QR, TLOC // 2], BF,
                             addr_space="Shared").ap() for i in range(2)]

    groups = [list(range(NCORES))]

    with tile.TileContext(nc) as tc, ExitStack() as stk:
        # ---------------- constants ----------------
        const = stk.enter_context(tc.tile_pool(name="const", bufs=1))
        ident = const.tile([P, P], BF)
        make_identity(nc, ident)
        ones_tok = const.tile([1, TLOC], BF)
        nc.vector.memset(ones_tok, 1.0)
        ones_row = const.tile([1, P], BF)
        nc.vector.memset(ones_row, 1.0)
        ones_col = const.tile([P, 1], BF)
        nc.vector.memset(ones_col, 1.0)
        eps_t = const.tile([P, 1], F32)
        nc.vector.memset(eps_t, LN_EPS)

        # shared PSUM pool: A,B,C are 2-bank [128,1024] f32 tiles, D is a
        # 1-bank [128,512] double-buffered tile -> 8 banks total.
        psum = stk.enter_context(tc.tile_pool(name="psum", bufs=1, space="PSUM"))

        def big(tag):
            return psum.tile([P, 1024], F32, tag=tag, name="ps" + tag, bufs=1)

        # persistent weight pool; loads are emitted inside phase 1, gated on
        # the kv down-proj weights so they don't steal DMA bandwidth from the
        # startup-critical x / wkv transfers.
        wu = stk.enter_context(tc.tile_pool(name="wu", bufs=1))

        def load_w(dram, rows, cols):
            if rows < P:
                t = wu.tile([rows, 1, cols], BF, name="w_" + dram.tensor.name)
                nc.scalar.dma_start(t[:, 0, :], dram[:, :])
                return t
            t = wu.tile([P, rows // P, cols], BF, name="w_" + dram.tensor.name)
            nc.scalar.dma_start(t[:], dram.rearrange("(k p) n -> p k n", p=P))
            return t

        # ------------- phase 1: down-proj + LN + transpose, kv first -------------
        with (
            tc.tile_pool(name="p1x", bufs=1) as xpool,
            tc.tile_pool(name="p1w", bufs=2) as wpool,
            tc.tile_pool(name="p1c", bufs=1) as cpool,
            tc.tile_pool(name="p1z", bufs=1) as zpool,
            tc.tile_pool(name="p1s", bufs=2) as spool,
        ):
            x_sl = [xpool.tile([P, 4, TLOC], BF, tag=f"x{g}", name=f"x{g}")
                    for g in range(4)]
            xr = xt.rearrange("(k p) m -> p k m", p=P)

            def chunk_w(wdram, col0, also_x=False):
                # weights (and optionally x) in 4 k-slabs so the first
                # matmuls start after ~1MB of DMA, not 4MB
                wr = wdram[:, col0:col0 + 512].rearrange("(k p) n -> p k n", p=P)
                wsl = []
                for g in range(4):
                    w = wpool.tile([P, 4, 512], BF, tag=f"w{g}", name=f"w{g}")
                    nc.sync.dma_start(w[:], wr[:, 4 * g:4 * g + 4, :])
                    if also_x:
                        nc.sync.dma_start(x_sl[g][:], xr[:, 4 * g:4 * g + 4, :])
                    wsl.append(w)
                return wsl

            z_kv = zpool.tile([P, KKV, TLOC], BF, name="z_kv")
            z_q = zpool.tile([P, KQ, TLOC], BF, name="z_q")
            cq_bf = cpool.tile([P, MT, QR], BF, name="cq_bf")
            ssum_kv = cpool.tile([P, MT], F32, name="ssum_kv")
            ssq_kv = cpool.tile([P, MT], F32, name="ssq_kv")
            ssum_q = cpool.tile([P, MT, 3], F32, name="ssum_q")
            ssq_q = cpool.tile([P, MT, 3], F32, name="ssq_q")

            def down_mms(wsl, slots, post=None):
                for k in range(KX):
                    for m in range(MT):
                        nc.tensor.matmul(
                            slots[m], x_sl[k // 4][:, k % 4, m * P:(m + 1) * P],
                            wsl[k // 4][:, k % 4, :],
                            start=(k == 0), stop=(k == KX - 1))
                    if post and k in post:
                        post[k]()

            def slots_ab():
                a, b = big("A"), big("B")
                return [a[:, 0:512], a[:, 512:1024], b[:, 0:512], b[:, 512:1024]]

            def slots_cd():
                c = big("C")
                d0 = psum.tile([P, 512], F32, tag="D", name="psD", bufs=2)
                d1 = psum.tile([P, 512], F32, tag="D", name="psD", bufs=2)
                return [c[:, 0:512], c[:, 512:1024], d0[:], d1[:]]

            def stats(slots, sum_ap, sq_ap, copy_to=None):
                for m in range(MT):
                    nc.vector.reduce_sum(sum_ap(m), slots[m], axis=AX.X)
                    scr = spool.tile([P, 512], BF, tag="scr", name="scr")
                    nc.scalar.activation(scr[:], slots[m], ACT.Square,
                                         accum_out=sq_ap(m))
                    if copy_to is not None:
                        nc.vector.tensor_copy(copy_to(m), slots[m])

            def ln_inv(ssum_ap, ssq_ap, rank, tagsuf):
                mu = spool.tile([P, 1], F32, tag="mu" + tagsuf, name="mu")
                nc.vector.tensor_scalar_mul(mu, ssum_ap, 1.0 / rank)
                musq = spool.tile([P, 1], F32, tag="ms" + tagsuf, name="musq")
                nc.vector.tensor_mul(musq, mu, mu)
                var = spool.tile([P, 1], F32, tag="va" + tagsuf, name="var")
                nc.vector.tensor_scalar_mul(var, ssq_ap, 1.0 / rank)
                nc.vector.tensor_sub(var, var, musq)
                sd = spool.tile([P, 1], F32, tag="sd" + tagsuf, name="sd")
                nc.scalar.activation(sd, var, ACT.Sqrt, bias=eps_t[:])
                inv = spool.tile([P, 1], F32, tag="iv" + tagsuf, name="inv")
                nc.vector.reciprocal(inv, sd)
                return mu, inv

            def transpose_group(cn_ap, zdst, f0, m, tagc):
                # 4 PE transposes into one psum group + a single batched copy
                tpsg = psum.tile([P, 4, P], BF, tag=tagc, name="tpsg", bufs=1)
                for f in range(4):
                    nc.tensor.transpose(tpsg[:, f, :],
                                        cn_ap[:, (f0 + f) * P:(f0 + f + 1) * P],
                                        ident)
                nc.vector.tensor_copy(zdst[:, f0:f0 + 4, m * P:(m + 1) * P],
                                      tpsg[:])

            # --- kv chunk (psum A,B); x DMAs interleaved with kv w slabs ---
            kv_slots = slots_ab()
            wkv_sl = chunk_w(wkv_down, 0, also_x=True)
            down_mms(wkv_sl, kv_slots)
            stats(kv_slots, lambda m: ssum_kv[:, m:m + 1],
                  lambda m: ssq_kv[:, m:m + 1])

            # kv LN math (DVE/ACT); overlaps q chunk 0's matmuls below
            cn_kv = []
            for m in range(MT):
                mu, inv = ln_inv(ssum_kv[:, m:m + 1], ssq_kv[:, m:m + 1],
                                 KVR, "kv")
                cnm = spool.tile([P, KVR], BF, tag="cnkv", name="cn_kv")
                nc.vector.tensor_scalar(cnm[:], kv_slots[m], scalar1=mu,
                                        scalar2=inv, op0=OP.subtract,
                                        op1=OP.mult)
                cn_kv.append(cnm)

            # --- q chunk 0 (psum C,D), kv transposes interleaved mid-loop ---
            def kv_tr(ms):
                def fn():
                    for m in ms:
                        transpose_group(cn_kv[m], z_kv, 0, m,
                                        "A" if m % 2 == 0 else "B")
                return fn
            q0_slots = slots_cd()
            wq0_sl = chunk_w(wq_down, 0)
            down_mms(wq0_sl, q0_slots, post={9: kv_tr((0, 1)),
                                             12: kv_tr((2, 3))})
            nc.sync.dma_start(
                agi_kv.rearrange("(k p) m -> p k m", p=P), z_kv[:])
            nc.gpsimd.collective_compute(
                "AllGather", OP.bypass, ins=[agi_kv[:]], outs=[ago_kv[:]],
                replica_groups=groups)
            stats(q0_slots, lambda m: ssum_q[:, m, 0:1],
                  lambda m: ssq_q[:, m, 0:1],
                  copy_to=lambda m: cq_bf[:, m, 0:512])

            # --- q chunk 1 (A,B) ---
            q1_slots = slots_ab()
            wq1_sl = chunk_w(wq_down, 512)
            down_mms(wq1_sl, q1_slots)
            stats(q1_slots, lambda m: ssum_q[:, m, 1:2],
                  lambda m: ssq_q[:, m, 1:2],
                  copy_to=lambda m: cq_bf[:, m, 512:1024])

            # --- q chunk 2 (C,D), m-major so each token-tile's LayerNorm +
            # transposes run under the next tile's matmuls ---
            q2_slots = slots_cd()
            wq2_sl = chunk_w(wq_down, 1024)

            # persistent weight / rope-table preloads on the scalar queue,
            # gated behind the last down-proj weight slab so they never
            # steal DMA bandwidth from the startup-critical path; ordered
            # by first use (kv up-proj path first).
            gate = spool.tile([1, 1], BF, tag="gate", name="gate")
            nc.scalar.copy(gate[:], wq2_sl[3][0:1, 0, 0:1])
            gku_t = load_w(gk_up, KVR, NQ)
            gkr_t = load_w(gk_rope, KVR, NR)
            gkrr_t = load_w(gk_rope_rot, KVR, NR)
            gvu_t = load_w(gv_up, KVR, NQ)
            cos_sb = wu.tile([NR, T], BF, name="cos_sb")
            nc.scalar.dma_start(cos_sb[:], cos_t[:, :])
            sin_sb = wu.tile([NR, T], BF, name="sin_sb")
            nc.scalar.dma_start(sin_sb[:], sin_t[:, :])
            gqu_t = load_w(gq_up, QR, NQ)
            gqr_t = load_w(gq_rope, QR, NR)
            gqrr_t = load_w(gq_rope_rot, QR, NR)
            wout_t = load_w(wout, NQ, D)
            bias_w = {}
            if has_bias:
                bias_w = dict(bqu=load_w(bq_up, 1, NQ),
                              bqr=load_w(bq_rope, 1, NR),
                              bqrr=load_w(bq_rope_rot, 1, NR),
                              bku=load_w(bk_up, 1, NQ),
                              bkr=load_w(bk_rope, 1, NR),
                              bkrr=load_w(bk_rope_rot, 1, NR),
                              bvu=load_w(bv_up, 1, NQ))

            prev_tr = None
            for m in range(MT):
                for k in range(KX):
                    nc.tensor.matmul(
                        q2_slots[m], x_sl[k // 4][:, k % 4, m * P:(m + 1) * P],
                        wq2_sl[k // 4][:, k % 4, :],
                        start=(k == 0), stop=(k == KX - 1))
                if prev_tr is not None:
                    prev_tr()
                # stats for this tile, then the full-rank LN + normalize
                nc.vector.reduce_sum(ssum_q[:, m, 2:3], q2_slots[m], axis=AX.X)
                scr = spool.tile([P, 512], BF, tag="scr", name="scr")
                nc.scalar.activation(scr[:], q2_slots[m], ACT.Square,
                                     accum_out=ssq_q[:, m, 2:3])
                nc.vector.tensor_copy(cq_bf[:, m, 1024:1536], q2_slots[m])
                st = spool.tile([P, 1], F32, tag="stq", name="st")
                nc.vector.tensor_add(st, ssum_q[:, m, 0:1], ssum_q[:, m, 1:2])
                nc.vector.tensor_add(st, st, ssum_q[:, m, 2:3])
                sq = spool.tile([P, 1], F32, tag="sqq", name="sq")
                nc.vector.tensor_add(sq, ssq_q[:, m, 0:1], ssq_q[:, m, 1:2])
                nc.vector.tensor_add(sq, sq, ssq_q[:, m, 2:3])
                mu, inv = ln_inv(st[:], sq[:], QR, "q")
                cnq = spool.tile([P, QR], BF, tag="cnq", name="cn_q")
                nc.vector.tensor_scalar(cnq[:], cq_bf[:, m, :], scalar1=mu,
                                        scalar2=inv, op0=OP.subtract,
                                        op1=OP.mult)

                def make_tr(cn_ap, mm):
                    def fn():
                        for g in range(3):
                            transpose_group(cn_ap, z_q, 4 * g, mm,
                                            "A" if (mm * 3 + g) % 2 == 0
                                            else "B")
                    return fn
                prev_tr = make_tr(cnq, m)
            prev_tr()

            for i in range(2):
                nc.sync.dma_start(
                    agi_qh[i].rearrange("(k p) m -> p k m", p=P),
                    z_q[:, :, i * 256:(i + 1) * 256])
                nc.gpsimd.collective_compute(
                    "AllGather", OP.bypass, ins=[agi_qh[i][:]],
                    outs=[ago_qh[i][:]], replica_groups=groups)

        if phases < 2:
            out0 = const.tile([P, 512], BF)
            nc.vector.memset(out0, 0.0)
            nc.sync.dma_start(out_p[0:P, 0:512], out0[:])

        # ---------------- phase 2: up-projections + rope ----------------
        qkv = stk.enter_context(tc.tile_pool(name="qkv", bufs=1))
        qc_sb = [qkv.tile([P, T], BF, tag=f"qc{m}", name=f"qc{m}")
                 for m in range(HLOC)]
        kc_sb = [qkv.tile([P, T], BF, tag=f"kc{m}", name=f"kc{m}")
                 for m in range(HLOC)]
        qr_sb = qkv.tile([NR, T], BF, tag="qr", name="qr")
        kr_sb = qkv.tile([NR, T], BF, tag="kr", name="kr")
        v_sb = qkv.tile([P, T // P, NQ], BF, tag="v", name="v")

        rot = {"i": 0}

        def next_half():
            i = rot["i"] % 6
            rot["i"] += 1
            t = big("ABC"[i // 2])
            return t[:, (i % 2) * 512:(i % 2) * 512 + 512]

        def proj(zt, nk, wt, mcol0, btile, w=TLOC):
            ps = next_half()[:, 0:w]
            for k in range(nk):
                nc.tensor.matmul(
                    ps, wt[:, k, mcol0:mcol0 + P], zt[:, k, :],
                    start=(k == 0), stop=(k == nk - 1 and btile is None))
            if btile is not None:
                nc.tensor.matmul(ps, btile[:, 0, mcol0:mcol0 + P],
                                 ones_tok[:, 0:w], start=False, stop=True)
            return ps

        with (
            tc.tile_pool(name="p2z", bufs=1) as zpool2,
            tc.tile_pool(name="p2zq", bufs=2) as zqpool,
            tc.tile_pool(name="p2t", bufs=2) as rtmp,
        ):
            def rope_pair(zt, nk, wr, wrr, br, brr, dst, csl, w=TLOC):
                psa = proj(zt, nk, wr, 0, br, w)
                psb = proj(zt, nk, wrr, 0, brr, w)
                t1 = rtmp.tile([NR, TLOC], BF, tag="t1", name="t1")
                nc.vector.tensor_mul(t1[:, 0:w], psa, cos_sb[:, csl])
                t2 = rtmp.tile([NR, TLOC], BF, tag="t2", name="t2")
                nc.vector.tensor_mul(t2[:, 0:w], psb, sin_sb[:, csl])
                nc.vector.tensor_add(dst[:, csl], t1[:, 0:w], t2[:, 0:w])

            # kv path; gathered latents staged in 1MB chunk-pair groups so
            # the first projections start right after the AllGather lands
            zkv_g = []
            for grp in range(4):
                t = zpool2.tile([P, 8, TLOC], BF, tag=f"zkv{grp}",
                                name=f"zkv{grp}")
                nc.sync.dma_start(
                    t[:], ago_kv[grp * 2 * KVR:(grp + 1) * 2 * KVR, :]
                    .rearrange("(k p) m -> p k m", p=P))
                zkv_g.append(t)

            for c in range(NCORES):
                csl = slice(c * TLOC, (c + 1) * TLOC)
                zkv = zkv_g[c // 2][:, (c % 2) * KKV:(c % 2 + 1) * KKV, :]
                for m in range(HLOC):
                    ps = proj(zkv, KKV, gku_t, m * P,
                              bias_w.get("bku"))
                    nc.scalar.copy(kc_sb[m][:, csl], ps)
                rope_pair(zkv, KKV, gkr_t, gkrr_t,
                          bias_w.get("bkr"), bias_w.get("bkrr"), kr_sb, csl)
                for mp in range(2):  # v: two psum halves, 2 token tiles each
                    ph = next_half()
                    for j in range(2):
                        mt = 2 * mp + j
                        for k in range(KKV):
                            nc.tensor.matmul(
                                ph[:, j * 256:j * 256 + 256],
                                zkv[:, k, mt * P:(mt + 1) * P], gvu_t[:, k, :],
                                start=(k == 0),
                                stop=(k == KKV - 1 and not has_bias))
                        if has_bias:
                            nc.tensor.matmul(ph[:, j * 256:j * 256 + 256],
                                             ones_row[:, :P],
                                             bias_w["bvu"][:, 0, :],
                                             start=False, stop=True)
                    ti = c * MT + 2 * mp
                    nc.scalar.copy(v_sb[:, ti:ti + 2, :], ph)

            # q path: token half A for all chunks (rides under half-B's
            # AllGather), then half B
            for half in range(2):
                for c in range(NCORES):
                    base = c * TLOC + half * 256
                    csl = slice(base, base + 256)
                    zq = zqpool.tile([P, KQ, 256], BF, tag="zq", name="zq")
                    nc.sync.dma_start(
                        zq[:], ago_qh[half][c * QR:(c + 1) * QR, :]
                        .rearrange("(k p) m -> p k m", p=P))
                    for m in range(HLOC):
                        ps = proj(zq, KQ, gqu_t, m * P, bias_w.get("bqu"), 256)
                        nc.scalar.copy(qc_sb[m][:, csl], ps)
                    rope_pair(zq, KQ, gqr_t, gqrr_t,
                              bias_w.get("bqr"), bias_w.get("bqrr"),
                              qr_sb, csl, 256)

        # ------------- phase 3: pipelined attention + interleaved out-proj -------------
        apool = stk.enter_context(tc.tile_pool(name="p3s", bufs=1))
        ppool = stk.enter_context(tc.tile_pool(name="p3p", bufs=1))
        opool = stk.enter_context(tc.tile_pool(name="p3o", bufs=2))

        acc_t = [apool.tile([P, 1024], BF, tag=f"acc{i}", name=f"acc{i}")
                 for i in range(2)]
        oT_sb = apool.tile([P, 1024], BF, tag="oT", name="oT")
        o_raw = apool.tile([P, 1024], BF, tag="oraw", name="o_raw")
        denf = apool.tile([1, 1024], F32, tag="denf", name="denf")
        rdf_f = apool.tile([1, 1024], F32, tag="rdff", name="rdf_f")
        rsc_t = apool.tile([1, 1024], F32, tag="rsc", name="rsc")
        rd_bf = apool.tile([1, 1024], BF, tag="rdbf", name="rd_bf")
        rb_bf = apool.tile([P, 1024], BF, tag="rbbf", name="rb_bf")

        def make_tail(st):
            """Return slot->emitters finishing block `st` (softmax denom,
            normalize, out-proj, DMA).  Emitted while the NEXT block's score
            stream keeps the PE busy; the denominator reduce/broadcast ride
            on cheap [1,512]-out PE matmuls so no engine ever waits long."""
            if st is None:
                return {}
            tok0 = st["tok0"]
            steps = {}

            def free_o():  # move raw PV sums out of psum so AV(next,0) can start
                nc.vector.tensor_copy(o_raw[:], st["o_ps"][:])
            steps[0] = [free_o]

            def den_mm():  # per-query column sums of exp, both heads
                for h in range(HLOC):
                    dps = psum.tile([1, 512], F32, tag="D", name="dps", bufs=2)
                    nc.tensor.matmul(dps, ones_col[:],
                                     st["acc"][:, h * 512:(h + 1) * 512],
                                     start=True, stop=True)
                    nc.vector.tensor_copy(denf[:, h * 512:(h + 1) * 512], dps)
            steps[1] = [den_mm]

            def recip():
                nc.vector.reciprocal_approx_accurate(rdf_f[:], denf[:], rsc_t[:])
                nc.vector.tensor_copy(rd_bf[:], rdf_f[:])
            steps[2] = [recip]

            def bc():  # broadcast 1/den across the 128 head-dim partitions
                for h in range(HLOC):
                    bps = psum.tile([P, 512], F32, tag="D", name="bps", bufs=2)
                    nc.tensor.matmul(bps, ones_row[:],
                                     rd_bf[:, h * 512:(h + 1) * 512],
                                     start=True, stop=True)
                    nc.vector.tensor_copy(rb_bf[:, h * 512:(h + 1) * 512], bps)
            steps[3] = [bc]

            def norm():
                nc.vector.tensor_mul(oT_sb[:], o_raw[:], rb_bf[:])
            steps[4] = [norm]

            state = {"ot": None}

            def op_group(g):
                def fn():
                    mtl, nch = g // 4, g % 4
                    if nch == 0:
                        state["ot"] = opool.tile([P, D], BF, tag="ot",
                                                 name="ot")
                    po = psum.tile([P, 512], F32, tag="D", name="po", bufs=2)
                    for h in range(HLOC):
                        nc.tensor.matmul(
                            po,
                            oT_sb[:, h * 512 + mtl * P:h * 512 + (mtl + 1) * P],
                            wout_t[:, h, nch * 512:(nch + 1) * 512],
                            start=(h == 0), stop=(h == HLOC - 1))
                    nc.vector.tensor_copy(
                        state["ot"][:, nch * 512:(nch + 1) * 512], po)
                    if nch == 3:
                        r0 = tok0 + mtl * P
                        nc.sync.dma_start(out_p[r0:r0 + P, :], state["ot"][:])
                return fn

            # 16 out-proj groups spread over slots 5..15 and the block end
            done = 0
            for slot in range(5, 16):
                want = (slot - 4) * 16 // 12
                lst = steps.setdefault(slot, [])
                while done < want:
                    lst.append(op_group(done))
                    done += 1
            lst = steps.setdefault("end", [])
            while done < 16:
                lst.append(op_group(done))
                done += 1
            return steps

        def emit_av(o_ps, s, bi, pts):
            pt = pts[s]
            for h in range(HLOC):
                nc.tensor.matmul(
                    o_ps[:, h * 512:(h + 1) * 512],
                    v_sb[:, bi * NKT + s, h * P:(h + 1) * P],
                    pt[:, h * 512:(h + 1) * 512],
                    start=(s == 0), stop=(s == NKT - 1))

        prev = None
        for blk in range(NBLK):
            bi, qch = blk // (S // 512), blk % (S // 512)
            tok0 = bi * S + qch * 512
            tsl = slice(tok0, tok0 + 512)
            acc = acc_t[blk % 2]
            o_ps = big("C")
            tail = make_tail(prev)
            pts = {}
            for s in range(NKT):
                St = big("AB"[s % 2])
                kt0 = bi * S + s * P
                for h in range(HLOC):
                    nc.tensor.matmul(
                        St[:, h * 512:(h + 1) * 512],
                        kc_sb[h][:, kt0:kt0 + P], qc_sb[h][:, tsl],
                        start=True, stop=False)
                for h in range(HLOC):
                    nc.tensor.matmul(
                        St[:, h * 512:(h + 1) * 512],
                        kr_sb[h * RD:(h + 1) * RD, kt0:kt0 + P],
                        qr_sb[h * RD:(h + 1) * RD, tsl],
                        start=False, stop=True)
                pt = ppool.tile([P, 1024], BF, tag=f"pt{s % 4}", name="pt")
                nc.scalar.activation(pt[:], St[:], ACT.Exp)
                if s == 0:
                    nc.vector.tensor_copy(acc[:], pt[:])
                else:
                    with nc.allow_low_precision(
                            reason="softmax denom accum ok in bf16"):
                        nc.vector.tensor_add(acc[:], acc[:], pt[:])
                pts[s] = pt
                for fn in tail.get(s, []):
                    fn()
                if s >= AV_LAG:
                    emit_av(o_ps, s - AV_LAG, bi, pts)
            for s2 in range(NKT - AV_LAG, NKT):
                emit_av(o_ps, s2, bi, pts)
            for fn in tail.get("end", []):
                fn()
            prev = {"tok0": tok0, "acc": acc, "o_ps": o_ps}

        # tail of the final block (PE idles briefly on the denom chain)
        tail = make_tail(prev)
        for slot in list(range(0, 16)) + ["end"]:
            for fn in tail.get(slot, []):
                fn()

    nc.compile()
    return nc


_BUILD_CACHE = {}


def _get_nc(has_bias: bool):
    if has_bias not in _BUILD_CACHE:
        _BUILD_CACHE[has_bias] = build(has_bias)
    return _BUILD_CACHE[has_bias]


def _bf(a):
    return np.ascontiguousarray(a).astype(ml_dtypes.bfloat16)


def _prep_in_maps(x, Wq_down, q_gamma, q_beta, Wq_up, Wq_rope,
                  Wkv_down, kv_gamma, kv_beta, Wk_up, Wv_up, Wk_rope, Wout):
    x = np.asarray(x, dtype=np.float32)
    xT = np.ascontiguousarray(x.reshape(T, D).T)  # [D, T]

    # rope rotate-half permutation, per 64-dim head block (2 local heads)
    Pi1 = np.zeros((RD, RD), np.float32)
    for i in range(RD // 2):
        Pi1[RD // 2 + i, i] = -1.0
        Pi1[i, RD // 2 + i] = 1.0
    Pi = np.zeros((NR, NR), np.float32)
    Pi[:RD, :RD] = Pi1
    Pi[RD:, RD:] = Pi1

    # rope tables, feature-major, duplicated for the 2 local heads
    inv_freq = 1.0 / (10000.0 ** (np.arange(0, RD, 2, dtype=np.float32) / RD))
    pos = (np.arange(T) % S).astype(np.float32)
    freqs = pos[:, None] * inv_freq[None, :]          # [T, 32]
    emb = np.concatenate([freqs, freqs], axis=1)      # [T, 64]
    cosT = np.ascontiguousarray(np.cos(emb).T)        # [64, T]
    sinT = np.ascontiguousarray(np.sin(emb).T)
    cos2 = np.concatenate([cosT, cosT], axis=0)       # [128, T]
    sin2 = np.concatenate([sinT, sinT], axis=0)

    q_gamma = np.asarray(q_gamma, np.float32)
    q_beta = np.asarray(q_beta, np.float32)
    kv_gamma = np.asarray(kv_gamma, np.float32)
    kv_beta = np.asarray(kv_beta, np.float32)
    has_bias = bool(np.any(q_beta) or np.any(kv_beta))

    Wq_up_h = np.asarray(Wq_up, np.float32).reshape(QR, H, HD)
    Wq_rope_h = np.asarray(Wq_rope, np.float32).reshape(QR, H, RD)
    Wk_up_h = np.asarray(Wk_up, np.float32).reshape(KVR, H, HD)
    Wk_rope_h = np.asarray(Wk_rope, np.float32).reshape(KVR, H, RD)
    Wv_up_h = np.asarray(Wv_up, np.float32).reshape(KVR, H, HD)
    Wout_h = np.asarray(Wout, np.float32).reshape(H, HD, D)

    in_maps = []
    for c in range(NCORES):
        hs = slice(HLOC * c, HLOC * (c + 1))
        wq_up_s = Wq_up_h[:, hs].reshape(QR, NQ)
        wq_rope_s = Wq_rope_h[:, hs].reshape(QR, NR)
        wk_up_s = Wk_up_h[:, hs].reshape(KVR, NQ)
        wk_rope_s = Wk_rope_h[:, hs].reshape(KVR, NR)
        wv_up_s = Wv_up_h[:, hs].reshape(KVR, NQ)
        wout_s = Wout_h[hs].reshape(NQ, D)

        gq_up = q_gamma[:, None] * wq_up_s * SCALE
        gq_rope = q_gamma[:, None] * wq_rope_s * SCALE
        gk_up = kv_gamma[:, None] * wk_up_s
        gk_rope = kv_gamma[:, None] * wk_rope_s
        gv_up = kv_gamma[:, None] * wv_up_s

        m = {
            "xt": _bf(xT[:, c * TLOC:(c + 1) * TLOC]),
            "wq_down": _bf(Wq_down),
            "wkv_down": _bf(Wkv_down),
            "gq_up": _bf(gq_up),
            "gq_rope": _bf(gq_rope),
            "gq_rope_rot": _bf(gq_rope @ Pi),
            "gk_up": _bf(gk_up),
            "gk_rope": _bf(gk_rope),
            "gk_rope_rot": _bf(gk_rope @ Pi),
            "gv_up": _bf(gv_up),
            "wout": _bf(wout_s),
            "cos_t": _bf(cos2),
            "sin_t": _bf(sin2),
        }
        if has_bias:
            m["bq_up"] = _bf((q_beta @ wq_up_s * SCALE)[None, :])
            bqr = q_beta @ wq_rope_s * SCALE
            m["bq_rope"] = _bf(bqr[None, :])
            m["bq_rope_rot"] = _bf((bqr @ Pi)[None, :])
            m["bk_up"] = _bf((kv_beta @ wk_up_s)[None, :])
            bkr = kv_beta @ wk_rope_s
            m["bk_rope"] = _bf(bkr[None, :])
            m["bk_rope_rot"] = _bf((bkr @ Pi)[None, :])
            m["bv_up"] = _bf((kv_beta @ wv_up_s)[None, :])
        in_maps.append(m)
    return in_maps, has_bias


def kernel(**inputs):
    in_maps, has_bias = _prep_in_maps(**inputs)
    nc = _get_nc(has_bias)
    res = run_bass_kernel_spmd(nc, in_maps, list(range(NCORES)))
    out = res.results[0]["out_p"].astype(np.float32)
    for c in range(1, NCORES):
        out = out + res.results[c]["out_p"].astype(np.float32)
    return out.reshape(B, S, D)


# revision 50
# speedup vs baseline: 1.0519x; 1.0519x over previous
"""Multi-Head Latent Attention (MLA) forward pass on 8 Trainium2 NeuronCores.

Sharding: num_heads tensor-parallel (2 heads/core) for up-projections,
attention and out-proj; the low-rank down-projections + LayerNorm are
token-parallel (512 tokens/core) followed by on-device AllGathers of the
bf16 latents (kv first, overlapped with the q path). Per-core partial
outputs (out-proj with input-dim-sliced Wout, bf16) are summed on the host.

Schedule: a tiny warmup collective absorbs the CC barrier/cold-start;
the kv AllGather is triggered as soon as the kv latents are normalized
(its LN runs while the q down-proj matmuls keep PE busy); the q AllGather
rides under the kv up-projection path.  Attention is software-pipelined:
scores for key-tile s+3 are issued before the PV matmuls of tile s, the
exp/row-sum run batched over both local heads ([128,1024] tiles), the
softmax denominator is reduced on the (otherwise idle) GpSimd engine,
and the out-projection of block b-1 is interleaved into block b's score
stream so the PE never drains.

Self-contained: hardcodes all shapes from the problem spec.
"""

from contextlib import ExitStack

import numpy as np
import ml_dtypes

import concourse.bass as bass
import concourse.mybir as mybir
import concourse.tile as tile
from concourse import bacc
from concourse.bass_utils import run_bass_kernel_spmd
from concourse.masks import make_identity

# ---- problem dimensions (hardcoded) ----
NCORES = 8
P = 128
B = 2
S = 2048           # sequence length
T = B * S          # total tokens = 4096
D = 2048           # d_model
QR = 1536          # q rank
KVR = 512          # kv rank
H = 16             # heads
HD = 128           # head dim (content)
RD = 64            # rope dim
HLOC = H // NCORES # heads per core = 2
TLOC = T // NCORES # tokens per core = 512
NQ = HLOC * HD     # 256 per-core content out dims
NR = HLOC * RD     # 128 per-core rope out dims
SCALE = (HD + RD) ** -0.5
LN_EPS = 1e-5

BF = mybir.dt.bfloat16
F32 = mybir.dt.float32
AX = mybir.AxisListType
OP = mybir.AluOpType
ACT = mybir.ActivationFunctionType

NKT = S // P       # 16 key tiles per sequence
KQ = QR // P       # 12
KKV = KVR // P     # 4
KX = D // P        # 16
MT = TLOC // P     # 4 token tiles per core
NBLK = B * (S // 512)  # 8 attention blocks of 512 q tokens
AV_LAG = 3         # PV matmuls trail the score matmuls by this many tiles


def build(has_bias: bool, phases: int = 3):
    nc = bacc.Bacc("TRN2", target_bir_lowering=False, debug=False,
                   num_devices=NCORES, enable_asserts=False)

    def din(name, shape, dt=BF):
        return nc.dram_tensor(name, shape, dt, kind="ExternalInput").ap()

    xt = din("xt", [D, TLOC])
    wq_down = din("wq_down", [D, QR])
    wkv_down = din("wkv_down", [D, KVR])
    gq_up = din("gq_up", [QR, NQ])
    gq_rope = din("gq_rope", [QR, NR])
    gk_up = din("gk_up", [KVR, NQ])
    gk_rope = din("gk_rope", [KVR, NR])
    gv_up = din("gv_up", [KVR, NQ])
    wout = din("wout", [NQ, D])
    cos_t = din("cos_t", [NR, T])
    sin_t = din("sin_t", [NR, T])  # rows d<32 of each 64-block pre-negated
    if has_bias:
        bq_up = din("bq_up", [1, NQ])
        bq_rope = din("bq_rope", [1, NR])
        bk_up = din("bk_up", [1, NQ])
        bk_rope = din("bk_rope", [1, NR])
        bv_up = din("bv_up", [1, NQ])
    out_p = nc.dram_tensor("out_p", [T, D], BF, kind="ExternalOutput").ap()

    agi_kv = nc.dram_tensor("agi_kv", [KVR, TLOC], BF).ap()
    ago_kv = nc.dram_tensor("ago_kv", [NCORES * KVR, TLOC], BF,
                            addr_space="Shared").ap()
    agi_qh = [nc.dram_tensor(f"agi_q{i}", [QR, TLOC // 2], BF).ap()
              for i in range(2)]
    ago_qh = [nc.dram_tensor(f"ago_q{i}", [NCORES * QR, TLOC // 2], BF,
                             addr_space="Shared").ap() for i in range(2)]

    groups = [list(range(NCORES))]

    with tile.TileContext(nc) as tc, ExitStack() as stk:
        # ---------------- constants ----------------
        const = stk.enter_context(tc.tile_pool(name="const", bufs=1))
        ident = const.tile([P, P], BF)
        make_identity(nc, ident)
        ones_tok = const.tile([1, TLOC], BF)
        nc.vector.memset(ones_tok, 1.0)
        ones_row = const.tile([1, P], BF)
        nc.vector.memset(ones_row, 1.0)
        ones_col = const.tile([P, 1], BF)
        nc.vector.memset(ones_col, 1.0)
        eps_t = const.tile([P, 1], F32)
        nc.vector.memset(eps_t, LN_EPS)

        # shared PSUM pool: A,B,C are 2-bank [128,1024] f32 tiles, D is a
        # 1-bank [128,512] double-buffered tile -> 8 banks total.
        psum = stk.enter_context(tc.tile_pool(name="psum", bufs=1, space="PSUM"))

        def big(tag):
            return psum.tile([P, 1024], F32, tag=tag, name="ps" + tag, bufs=1)

        # persistent weight pool; loads are emitted inside phase 1, gated on
        # the kv down-proj weights so they don't steal DMA bandwidth from the
        # startup-critical x / wkv transfers.
        wu = stk.enter_context(tc.tile_pool(name="wu", bufs=1))

        def load_w(dram, rows, cols):
            if rows < P:
                t = wu.tile([rows, 1, cols], BF, name="w_" + dram.tensor.name)
                nc.scalar.dma_start(t[:, 0, :], dram[:, :])
                return t
            t = wu.tile([P, rows // P, cols], BF, name="w_" + dram.tensor.name)
            nc.scalar.dma_start(t[:], dram.rearrange("(k p) n -> p k n", p=P))
            return t

        # ------------- phase 1: down-proj + LN + transpose, kv first -------------
        with (
            tc.tile_pool(name="p1x", bufs=1) as xpool,
            tc.tile_pool(name="p1w", bufs=2) as wpool,
            tc.tile_pool(name="p1c", bufs=1) as cpool,
            tc.tile_pool(name="p1z", bufs=1) as zpool,
            tc.tile_pool(name="p1s", bufs=2) as spool,
        ):
            x_sl = [xpool.tile([P, 4, TLOC], BF, tag=f"x{g}", name=f"x{g}")
                    for g in range(4)]
            xr = xt.rearrange("(k p) m -> p k m", p=P)

            def chunk_w(wdram, col0, also_x=False):
                # weights (and optionally x) in 4 k-slabs so the first
                # matmuls start after ~1MB of DMA, not 4MB
                wr = wdram[:, col0:col0 + 512].rearrange("(k p) n -> p k n", p=P)
                wsl = []
                for g in range(4):
                    w = wpool.tile([P, 4, 512], BF, tag=f"w{g}", name=f"w{g}")
                    nc.sync.dma_start(w[:], wr[:, 4 * g:4 * g + 4, :])
                    if also_x:
                        nc.sync.dma_start(x_sl[g][:], xr[:, 4 * g:4 * g + 4, :])
                    wsl.append(w)
                return wsl

            z_kv = zpool.tile([P, KKV, TLOC], BF, name="z_kv")
            z_q = zpool.tile([P, KQ, TLOC], BF, name="z_q")
            cq_bf = cpool.tile([P, MT, QR], BF, name="cq_bf")
            ssum_kv = cpool.tile([P, MT], F32, name="ssum_kv")
            ssq_kv = cpool.tile([P, MT], F32, name="ssq_kv")
            ssum_q = cpool.tile([P, MT, 3], F32, name="ssum_q")
            ssq_q = cpool.tile([P, MT, 3], F32, name="ssq_q")

            def down_mms(wsl, slots, post=None):
                for k in range(KX):
                    for m in range(MT):
                        nc.tensor.matmul(
                            slots[m], x_sl[k // 4][:, k % 4, m * P:(m + 1) * P],
                            wsl[k // 4][:, k % 4, :],
                            start=(k == 0), stop=(k == KX - 1))
                    if post and k in post:
                        post[k]()

            def slots_ab():
                a, b = big("A"), big("B")
                return [a[:, 0:512], a[:, 512:1024], b[:, 0:512], b[:, 512:1024]]

            def slots_cd():
                c = big("C")
                d0 = psum.tile([P, 512], F32, tag="D", name="psD", bufs=2)
                d1 = psum.tile([P, 512], F32, tag="D", name="psD", bufs=2)
                return [c[:, 0:512], c[:, 512:1024], d0[:], d1[:]]

            def stats(slots, sum_ap, sq_ap, copy_to=None):
                for m in range(MT):
                    nc.vector.reduce_sum(sum_ap(m), slots[m], axis=AX.X)
                    scr = spool.tile([P, 512], BF, tag="scr", name="scr")
                    nc.scalar.activation(scr[:], slots[m], ACT.Square,
                                         accum_out=sq_ap(m))
                    if copy_to is not None:
                        nc.vector.tensor_copy(copy_to(m), slots[m])

            def ln_inv(ssum_ap, ssq_ap, rank, tagsuf):
                mu = spool.tile([P, 1], F32, tag="mu" + tagsuf, name="mu")
                nc.vector.tensor_scalar_mul(mu, ssum_ap, 1.0 / rank)
                musq = spool.tile([P, 1], F32, tag="ms" + tagsuf, name="musq")
                nc.vector.tensor_mul(musq, mu, mu)
                var = spool.tile([P, 1], F32, tag="va" + tagsuf, name="var")
                nc.vector.tensor_scalar_mul(var, ssq_ap, 1.0 / rank)
                nc.vector.tensor_sub(var, var, musq)
                sd = spool.tile([P, 1], F32, tag="sd" + tagsuf, name="sd")
                nc.scalar.activation(sd, var, ACT.Sqrt, bias=eps_t[:])
                inv = spool.tile([P, 1], F32, tag="iv" + tagsuf, name="inv")
                nc.vector.reciprocal(inv, sd)
                return mu, inv

            def transpose_group(cn_ap, zdst, f0, m, tagc):
                # 4 PE transposes into one psum group + a single batched copy
                tpsg = psum.tile([P, 4, P], BF, tag=tagc, name="tpsg", bufs=1)
                for f in range(4):
                    nc.tensor.transpose(tpsg[:, f, :],
                                        cn_ap[:, (f0 + f) * P:(f0 + f + 1) * P],
                                        ident)
                nc.vector.tensor_copy(zdst[:, f0:f0 + 4, m * P:(m + 1) * P],
                                      tpsg[:])

            # --- kv chunk (psum A,B); x DMAs interleaved with kv w slabs ---
            kv_slots = slots_ab()
            wkv_sl = chunk_w(wkv_down, 0, also_x=True)
            down_mms(wkv_sl, kv_slots)
            stats(kv_slots, lambda m: ssum_kv[:, m:m + 1],
                  lambda m: ssq_kv[:, m:m + 1])

            # kv LN math (DVE/ACT); overlaps q chunk 0's matmuls below
            cn_kv = []
            for m in range(MT):
                mu, inv = ln_inv(ssum_kv[:, m:m + 1], ssq_kv[:, m:m + 1],
                                 KVR, "kv")
                cnm = spool.tile([P, KVR], BF, tag="cnkv", name="cn_kv")
                nc.vector.tensor_scalar(cnm[:], kv_slots[m], scalar1=mu,
                                        scalar2=inv, op0=OP.subtract,
                                        op1=OP.mult)
                cn_kv.append(cnm)

            # --- q chunk 0 (psum C,D), kv transposes interleaved mid-loop ---
            def kv_tr(ms):
                def fn():
                    for m in ms:
                        transpose_group(cn_kv[m], z_kv, 0, m,
                                        "A" if m % 2 == 0 else "B")
                return fn
            q0_slots = slots_cd()
            wq0_sl = chunk_w(wq_down, 0)
            down_mms(wq0_sl, q0_slots, post={9: kv_tr((0, 1)),
                                             12: kv_tr((2, 3))})
            nc.sync.dma_start(
                agi_kv.rearrange("(k p) m -> p k m", p=P), z_kv[:])
            nc.gpsimd.collective_compute(
                "AllGather", OP.bypass, ins=[agi_kv[:]], outs=[ago_kv[:]],
                replica_groups=groups)
            stats(q0_slots, lambda m: ssum_q[:, m, 0:1],
                  lambda m: ssq_q[:, m, 0:1],
                  copy_to=lambda m: cq_bf[:, m, 0:512])

            # --- q chunk 1 (A,B) ---
            q1_slots = slots_ab()
            wq1_sl = chunk_w(wq_down, 512)
            down_mms(wq1_sl, q1_slots)
            stats(q1_slots, lambda m: ssum_q[:, m, 1:2],
                  lambda m: ssq_q[:, m, 1:2],
                  copy_to=lambda m: cq_bf[:, m, 512:1024])

            # --- q chunk 2 (C,D), m-major so each token-tile's LayerNorm +
            # transposes run under the next tile's matmuls ---
            q2_slots = slots_cd()
            wq2_sl = chunk_w(wq_down, 1024)

            # persistent weight / rope-table preloads on the scalar queue,
            # gated behind the last down-proj weight slab so they never
            # steal DMA bandwidth from the startup-critical path; ordered
            # by first use (kv up-proj path first).
            gate = spool.tile([1, 1], BF, tag="gate", name="gate")
            nc.scalar.copy(gate[:], wq2_sl[3][0:1, 0, 0:1])
            gku_t = load_w(gk_up, KVR, NQ)
            gkr_t = load_w(gk_rope, KVR, NR)
            gvu_t = load_w(gv_up, KVR, NQ)
            cos_sb = wu.tile([NR, T], BF, name="cos_sb")
            nc.scalar.dma_start(cos_sb[:], cos_t[:, :])
            sin_sb = wu.tile([NR, T], BF, name="sin_sb")
            nc.scalar.dma_start(sin_sb[:], sin_t[:, :])
            gqu_t = load_w(gq_up, QR, NQ)
            gqr_t = load_w(gq_rope, QR, NR)
            wout_t = load_w(wout, NQ, D)
            bias_w = {}
            if has_bias:
                bias_w = dict(bqu=load_w(bq_up, 1, NQ),
                              bqr=load_w(bq_rope, 1, NR),
                              bku=load_w(bk_up, 1, NQ),
                              bkr=load_w(bk_rope, 1, NR),
                              bvu=load_w(bv_up, 1, NQ))

            prev_tr = None
            for m in range(MT):
                for k in range(KX):
                    nc.tensor.matmul(
                        q2_slots[m], x_sl[k // 4][:, k % 4, m * P:(m + 1) * P],
                        wq2_sl[k // 4][:, k % 4, :],
                        start=(k == 0), stop=(k == KX - 1))
                if prev_tr is not None:
                    prev_tr()
                # stats for this tile, then the full-rank LN + normalize
                nc.vector.reduce_sum(ssum_q[:, m, 2:3], q2_slots[m], axis=AX.X)
                scr = spool.tile([P, 512], BF, tag="scr", name="scr")
                nc.scalar.activation(scr[:], q2_slots[m], ACT.Square,
                                     accum_out=ssq_q[:, m, 2:3])
                nc.vector.tensor_copy(cq_bf[:, m, 1024:1536], q2_slots[m])
                st = spool.tile([P, 1], F32, tag="stq", name="st")
                nc.vector.tensor_add(st, ssum_q[:, m, 0:1], ssum_q[:, m, 1:2])
                nc.vector.tensor_add(st, st, ssum_q[:, m, 2:3])
                sq = spool.tile([P, 1], F32, tag="sqq", name="sq")
                nc.vector.tensor_add(sq, ssq_q[:, m, 0:1], ssq_q[:, m, 1:2])
                nc.vector.tensor_add(sq, sq, ssq_q[:, m, 2:3])
                mu, inv = ln_inv(st[:], sq[:], QR, "q")
                cnq = spool.tile([P, QR], BF, tag="cnq", name="cn_q")
                nc.vector.tensor_scalar(cnq[:], cq_bf[:, m, :], scalar1=mu,
                                        scalar2=inv, op0=OP.subtract,
                                        op1=OP.mult)

                def make_tr(cn_ap, mm):
                    def fn():
                        for g in range(3):
                            transpose_group(cn_ap, z_q, 4 * g, mm,
                                            "A" if (mm * 3 + g) % 2 == 0
                                            else "B")
                    return fn
                prev_tr = make_tr(cnq, m)
            prev_tr()

            for i in range(2):
                nc.sync.dma_start(
                    agi_qh[i].rearrange("(k p) m -> p k m", p=P),
                    z_q[:, :, i * 256:(i + 1) * 256])
                nc.gpsimd.collective_compute(
                    "AllGather", OP.bypass, ins=[agi_qh[i][:]],
                    outs=[ago_qh[i][:]], replica_groups=groups)

        if phases < 2:
            out0 = const.tile([P, 512], BF)
            nc.vector.memset(out0, 0.0)
            nc.sync.dma_start(out_p[0:P, 0:512], out0[:])

        # ---------------- phase 2: up-projections + rope ----------------
        qkv = stk.enter_context(tc.tile_pool(name="qkv", bufs=1))
        qc_sb = [qkv.tile([P, T], BF, tag=f"qc{m}", name=f"qc{m}")
                 for m in range(HLOC)]
        kc_sb = [qkv.tile([P, T], BF, tag=f"kc{m}", name=f"kc{m}")
                 for m in range(HLOC)]
        qr_sb = qkv.tile([NR, T], BF, tag="qr", name="qr")
        kr_sb = qkv.tile([NR, T], BF, tag="kr", name="kr")
        v_sb = qkv.tile([P, T // P, NQ], BF, tag="v", name="v")

        rot = {"i": 0}

        def next_half():
            i = rot["i"] % 8
            rot["i"] += 1
            if i >= 6:
                return psum.tile([P, 512], F32, tag="D", name="psD2",
                                 bufs=2)[:]
            t = big("ABC"[i // 2])
            return t[:, (i % 2) * 512:(i % 2) * 512 + 512]

        def proj(zt, nk, wt, mcol0, btile, w=TLOC):
            ps = next_half()[:, 0:w]
            for k in range(nk):
                nc.tensor.matmul(
                    ps, wt[:, k, mcol0:mcol0 + P], zt[:, k, :],
                    start=(k == 0), stop=(k == nk - 1 and btile is None))
            if btile is not None:
                nc.tensor.matmul(ps, btile[:, 0, mcol0:mcol0 + P],
                                 ones_tok[:, 0:w], start=False, stop=True)
            return ps

        with (
            tc.tile_pool(name="p2z", bufs=1) as zpool2,
            tc.tile_pool(name="p2zq", bufs=2) as zqpool,
            tc.tile_pool(name="p2t", bufs=2) as rtmp,
        ):
            def rope_pair(zt, nk, wr, br, dst, csl, w=TLOC):
                # rotate-half computed with partition-shifted multiplies
                # against the sign-baked sin table (no second projection)
                psa = proj(zt, nk, wr, 0, br, w)
                t1 = rtmp.tile([NR, TLOC], BF, tag="t1", name="t1")
                nc.vector.tensor_mul(t1[:, 0:w], psa, cos_sb[:, csl])
                t2 = rtmp.tile([NR, TLOC], BF, tag="t2", name="t2")
                for h in range(HLOC):
                    b = h * RD
                    nc.vector.tensor_mul(t2[b:b + 32, 0:w],
                                         psa[b + 32:b + 64, :],
                                         sin_sb[b:b + 32, csl])
                    nc.vector.tensor_mul(t2[b + 32:b + 64, 0:w],
                                         psa[b:b + 32, :],
                                         sin_sb[b + 32:b + 64, csl])
                nc.vector.tensor_add(dst[:, csl], t1[:, 0:w], t2[:, 0:w])

            # kv path; gathered latents staged in 1MB chunk-pair groups so
            # the first projections start right after the AllGather lands
            zkv_g = []
            for grp in range(4):
                t = zpool2.tile([P, 8, TLOC], BF, tag=f"zkv{grp}",
                                name=f"zkv{grp}")
                nc.sync.dma_start(
                    t[:], ago_kv[grp * 2 * KVR:(grp + 1) * 2 * KVR, :]
                    .rearrange("(k p) m -> p k m", p=P))
                zkv_g.append(t)

            for c in range(NCORES):
                csl = slice(c * TLOC, (c + 1) * TLOC)
                zkv = zkv_g[c // 2][:, (c % 2) * KKV:(c % 2 + 1) * KKV, :]
                for m in range(HLOC):
                    ps = proj(zkv, KKV, gku_t, m * P,
                              bias_w.get("bku"))
                    nc.scalar.copy(kc_sb[m][:, csl], ps)
                rope_pair(zkv, KKV, gkr_t, bias_w.get("bkr"), kr_sb, csl)
                for mp in range(2):  # v: two psum halves, 2 token tiles each
                    ph = next_half()
                    for j in range(2):
                        mt = 2 * mp + j
                        for k in range(KKV):
                            nc.tensor.matmul(
                                ph[:, j * 256:j * 256 + 256],
                                zkv[:, k, mt * P:(mt + 1) * P], gvu_t[:, k, :],
                                start=(k == 0),
                                stop=(k == KKV - 1 and not has_bias))
                        if has_bias:
                            nc.tensor.matmul(ph[:, j * 256:j * 256 + 256],
                                             ones_row[:, :P],
                                             bias_w["bvu"][:, 0, :],
                                             start=False, stop=True)
                    ti = c * MT + 2 * mp
                    nc.scalar.copy(v_sb[:, ti:ti + 2, :], ph)

            # q path: token half A for all chunks (rides under half-B's
            # AllGather), then half B
            for half in range(2):
                for c in range(NCORES):
                    base = c * TLOC + half * 256
                    csl = slice(base, base + 256)
                    zq = zqpool.tile([P, KQ, 256], BF, tag="zq", name="zq")
                    nc.sync.dma_start(
                        zq[:], ago_qh[half][c * QR:(c + 1) * QR, :]
                        .rearrange("(k p) m -> p k m", p=P))
                    for m in range(HLOC):
                        ps = proj(zq, KQ, gqu_t, m * P, bias_w.get("bqu"), 256)
                        nc.scalar.copy(qc_sb[m][:, csl], ps)
                    rope_pair(zq, KQ, gqr_t, bias_w.get("bqr"),
                              qr_sb, csl, 256)

        # ------------- phase 3: pipelined attention + interleaved out-proj -------------
        apool = stk.enter_context(tc.tile_pool(name="p3s", bufs=1))
        ppool = stk.enter_context(tc.tile_pool(name="p3p", bufs=1))
        opool = stk.enter_context(tc.tile_pool(name="p3o", bufs=2))

        acc_t = [apool.tile([P, 1024], BF, tag=f"acc{i}", name=f"acc{i}")
                 for i in range(2)]
        oT_sb = apool.tile([P, 1024], BF, tag="oT", name="oT")
        o_raw = apool.tile([P, 1024], BF, tag="oraw", name="o_raw")
        denf = apool.tile([1, 1024], F32, tag="denf", name="denf")
        rdf_f = apool.tile([1, 1024], F32, tag="rdff", name="rdf_f")
        rsc_t = apool.tile([1, 1024], F32, tag="rsc", name="rsc")
        rd_bf = apool.tile([1, 1024], BF, tag="rdbf", name="rd_bf")
        rb_bf = apool.tile([P, 1024], BF, tag="rbbf", name="rb_bf")

        def make_tail(st):
            """Return slot->emitters finishing block `st` (softmax denom,
            normalize, out-proj, DMA).  Emitted while the NEXT block's score
            stream keeps the PE busy; the denominator reduce/broadcast ride
            on cheap [1,512]-out PE matmuls so no engine ever waits long."""
            if st is None:
                return {}
            tok0 = st["tok0"]
            steps = {}

            def free_o():  # move raw PV sums out of psum so AV(next,0) can start
                nc.vector.tensor_copy(o_raw[:], st["o_ps"][:])
            steps[0] = [free_o]

            def den_mm():  # per-query column sums of exp, both heads
                for h in range(HLOC):
                    dps = psum.tile([1, 512], F32, tag="D", name="dps", bufs=2)
                    nc.tensor.matmul(dps, ones_col[:],
                                     st["acc"][:, h * 512:(h + 1) * 512],
                                     start=True, stop=True)
                    nc.vector.tensor_copy(denf[:, h * 512:(h + 1) * 512], dps)
            steps[1] = [den_mm]

            def recip():
                nc.vector.reciprocal_approx_accurate(rdf_f[:], denf[:], rsc_t[:])
                nc.vector.tensor_copy(rd_bf[:], rdf_f[:])
            steps[2] = [recip]

            def bc():  # broadcast 1/den across the 128 head-dim partitions
                for h in range(HLOC):
                    bps = psum.tile([P, 512], F32, tag="D", name="bps", bufs=2)
                    nc.tensor.matmul(bps, ones_row[:],
                                     rd_bf[:, h * 512:(h + 1) * 512],
                                     start=True, stop=True)
                    nc.vector.tensor_copy(rb_bf[:, h * 512:(h + 1) * 512], bps)
            steps[3] = [bc]

            def norm():
                nc.vector.tensor_mul(oT_sb[:], o_raw[:], rb_bf[:])
            steps[4] = [norm]

            state = {"ot": None}

            def op_group(g):
                def fn():
                    mtl, nch = g // 4, g % 4
                    if nch == 0:
                        state["ot"] = opool.tile([P, D], BF, tag="ot",
                                                 name="ot")
                    po = psum.tile([P, 512], F32, tag="D", name="po", bufs=2)
                    for h in range(HLOC):
                        nc.tensor.matmul(
                            po,
                            oT_sb[:, h * 512 + mtl * P:h * 512 + (mtl + 1) * P],
                            wout_t[:, h, nch * 512:(nch + 1) * 512],
                            start=(h == 0), stop=(h == HLOC - 1))
                    nc.vector.tensor_copy(
                        state["ot"][:, nch * 512:(nch + 1) * 512], po)
                    if nch == 3:
                        r0 = tok0 + mtl * P
                        nc.sync.dma_start(out_p[r0:r0 + P, :], state["ot"][:])
                return fn

            # 16 out-proj groups spread over slots 5..15 and the block end
            done = 0
            for slot in range(5, 16):
                want = (slot - 4) * 16 // 12
                lst = steps.setdefault(slot, [])
                while done < want:
                    lst.append(op_group(done))
                    done += 1
            lst = steps.setdefault("end", [])
            while done < 16:
                lst.append(op_group(done))
                done += 1
            return steps

        def emit_av(o_ps, s, bi, pts):
            pt = pts[s]
            for h in range(HLOC):
                nc.tensor.matmul(
                    o_ps[:, h * 512:(h + 1) * 512],
                    v_sb[:, bi * NKT + s, h * P:(h + 1) * P],
                    pt[:, h * 512:(h + 1) * 512],
                    start=(s == 0), stop=(s == NKT - 1))

        prev = None
        for blk in range(NBLK):
            bi, qch = blk // (S // 512), blk % (S // 512)
            tok0 = bi * S + qch * 512
            tsl = slice(tok0, tok0 + 512)
            acc = acc_t[blk % 2]
            o_ps = big("C")
            tail = make_tail(prev)
            pts = {}
            for s in range(NKT):
                St = big("AB"[s % 2])
                kt0 = bi * S + s * P
                for h in range(HLOC):
                    nc.tensor.matmul(
                        St[:, h * 512:(h + 1) * 512],
                        kc_sb[h][:, kt0:kt0 + P], qc_sb[h][:, tsl],
                        start=True, stop=False)
                for h in range(HLOC):
                    nc.tensor.matmul(
                        St[:, h * 512:(h + 1) * 512],
                        kr_sb[h * RD:(h + 1) * RD, kt0:kt0 + P],
                        qr_sb[h * RD:(h + 1) * RD, tsl],
                        start=False, stop=True)
                pt = ppool.tile([P, 1024], BF, tag=f"pt{s % 4}", name="pt")
                nc.scalar.activation(pt[:], St[:], ACT.Exp)
                if s == 0:
                    nc.vector.tensor_copy(acc[:], pt[:])
                else:
                    with nc.allow_low_precision(
                            reason="softmax denom accum ok in bf16"):
                        nc.vector.tensor_add(acc[:], acc[:], pt[:])
                pts[s] = pt
                for fn in tail.get(s, []):
                    fn()
                if s >= AV_LAG:
                    emit_av(o_ps, s - AV_LAG, bi, pts)
            for s2 in range(NKT - AV_LAG, NKT):
                emit_av(o_ps, s2, bi, pts)
            for fn in tail.get("end", []):
                fn()
            prev = {"tok0": tok0, "acc": acc, "o_ps": o_ps}

        # tail of the final block (PE idles briefly on the denom chain)
        tail = make_tail(prev)
        for slot in list(range(0, 16)) + ["end"]:
            for fn in tail.get(slot, []):
                fn()

    nc.compile()
    return nc


_BUILD_CACHE = {}


def _get_nc(has_bias: bool):
    if has_bias not in _BUILD_CACHE:
        _BUILD_CACHE[has_bias] = build(has_bias)
    return _BUILD_CACHE[has_bias]


def _bf(a):
    return np.ascontiguousarray(a).astype(ml_dtypes.bfloat16)


def _prep_in_maps(x, Wq_down, q_gamma, q_beta, Wq_up, Wq_rope,
                  Wkv_down, kv_gamma, kv_beta, Wk_up, Wv_up, Wk_rope, Wout):
    x = np.asarray(x, dtype=np.float32)
    xT = np.ascontiguousarray(x.reshape(T, D).T)  # [D, T]

    # rope tables, feature-major, duplicated for the 2 local heads; the
    # first half of each 64-row block carries -sin so the kernel's
    # partition-shifted rotate-half multiplies need no sign op
    inv_freq = 1.0 / (10000.0 ** (np.arange(0, RD, 2, dtype=np.float32) / RD))
    pos = (np.arange(T) % S).astype(np.float32)
    freqs = pos[:, None] * inv_freq[None, :]          # [T, 32]
    emb = np.concatenate([freqs, freqs], axis=1)      # [T, 64]
    cosT = np.ascontiguousarray(np.cos(emb).T)        # [64, T]
    sinT = np.ascontiguousarray(np.sin(emb).T)
    sinT[:RD // 2, :] *= -1.0
    cos2 = np.concatenate([cosT, cosT], axis=0)       # [128, T]
    sin2 = np.concatenate([sinT, sinT], axis=0)

    q_gamma = np.asarray(q_gamma, np.float32)
    q_beta = np.asarray(q_beta, np.float32)
    kv_gamma = np.asarray(kv_gamma, np.float32)
    kv_beta = np.asarray(kv_beta, np.float32)
    has_bias = bool(np.any(q_beta) or np.any(kv_beta))

    Wq_up_h = np.asarray(Wq_up, np.float32).reshape(QR, H, HD)
    Wq_rope_h = np.asarray(Wq_rope, np.float32).reshape(QR, H, RD)
    Wk_up_h = np.asarray(Wk_up, np.float32).reshape(KVR, H, HD)
    Wk_rope_h = np.asarray(Wk_rope, np.float32).reshape(KVR, H, RD)
    Wv_up_h = np.asarray(Wv_up, np.float32).reshape(KVR, H, HD)
    Wout_h = np.asarray(Wout, np.float32).reshape(H, HD, D)

    in_maps = []
    for c in range(NCORES):
        hs = slice(HLOC * c, HLOC * (c + 1))
        wq_up_s = Wq_up_h[:, hs].reshape(QR, NQ)
        wq_rope_s = Wq_rope_h[:, hs].reshape(QR, NR)
        wk_up_s = Wk_up_h[:, hs].reshape(KVR, NQ)
        wk_rope_s = Wk_rope_h[:, hs].reshape(KVR, NR)
        wv_up_s = Wv_up_h[:, hs].reshape(KVR, NQ)
        wout_s = Wout_h[hs].reshape(NQ, D)

        gq_up = q_gamma[:, None] * wq_up_s * SCALE
        gq_rope = q_gamma[:, None] * wq_rope_s * SCALE
        gk_up = kv_gamma[:, None] * wk_up_s
        gk_rope = kv_gamma[:, None] * wk_rope_s
        gv_up = kv_gamma[:, None] * wv_up_s

        m = {
            "xt": _bf(xT[:, c * TLOC:(c + 1) * TLOC]),
            "wq_down": _bf(Wq_down),
            "wkv_down": _bf(Wkv_down),
            "gq_up": _bf(gq_up),
            "gq_rope": _bf(gq_rope),
            "gk_up": _bf(gk_up),
            "gk_rope": _bf(gk_rope),
            "gv_up": _bf(gv_up),
            "wout": _bf(wout_s),
            "cos_t": _bf(cos2),
            "sin_t": _bf(sin2),
        }
        if has_bias:
            m["bq_up"] = _bf((q_beta @ wq_up_s * SCALE)[None, :])
            m["bq_rope"] = _bf((q_beta @ wq_rope_s * SCALE)[None, :])
            m["bk_up"] = _bf((kv_beta @ wk_up_s)[None, :])
            m["bk_rope"] = _bf((kv_beta @ wk_rope_s)[None, :])
            m["bv_up"] = _bf((kv_beta @ wv_up_s)[None, :])
        in_maps.append(m)
    return in_maps, has_bias


def kernel(**inputs):
    in_maps, has_bias = _prep_in_maps(**inputs)
    nc = _get_nc(has_bias)
    res = run_bass_kernel_spmd(nc, in_maps, list(range(NCORES)))
    out = res.results[0]["out_p"].astype(np.float32)
    for c in range(1, NCORES):
        out = out + res.results[c]["out_p"].astype(np.float32)
    return out.reshape(B, S, D)


# revision 56
# speedup vs baseline: 1.0804x; 1.0271x over previous
"""Multi-Head Latent Attention (MLA) forward pass on 8 Trainium2 NeuronCores.

Sharding: num_heads tensor-parallel (2 heads/core) for up-projections,
attention and out-proj; the low-rank down-projections + LayerNorm are
token-parallel (512 tokens/core) followed by on-device AllGathers of the
bf16 latents (kv first, overlapped with the q path). Per-core partial
outputs (out-proj with input-dim-sliced Wout, bf16) are summed on the host.

Schedule: a tiny warmup collective absorbs the CC barrier/cold-start;
the kv AllGather is triggered as soon as the kv latents are normalized
(its LN runs while the q down-proj matmuls keep PE busy); the q AllGather
rides under the kv up-projection path.  Attention is software-pipelined:
scores for key-tile s+3 are issued before the PV matmuls of tile s, the
exp/row-sum run batched over both local heads ([128,1024] tiles), the
softmax denominator is reduced on the (otherwise idle) GpSimd engine,
and the out-projection of block b-1 is interleaved into block b's score
stream so the PE never drains.

Self-contained: hardcodes all shapes from the problem spec.
"""

from contextlib import ExitStack

import numpy as np
import ml_dtypes

import concourse.bass as bass
import concourse.mybir as mybir
import concourse.tile as tile
from concourse import bacc
from concourse.bass_utils import run_bass_kernel_spmd
from concourse.masks import make_identity

# ---- problem dimensions (hardcoded) ----
NCORES = 8
P = 128
B = 2
S = 2048           # sequence length
T = B * S          # total tokens = 4096
D = 2048           # d_model
QR = 1536          # q rank
KVR = 512          # kv rank
H = 16             # heads
HD = 128           # head dim (content)
RD = 64            # rope dim
HLOC = H // NCORES # heads per core = 2
TLOC = T // NCORES # tokens per core = 512
NQ = HLOC * HD     # 256 per-core content out dims
NR = HLOC * RD     # 128 per-core rope out dims
SCALE = (HD + RD) ** -0.5
LN_EPS = 1e-5

BF = mybir.dt.bfloat16
F32 = mybir.dt.float32
AX = mybir.AxisListType
OP = mybir.AluOpType
ACT = mybir.ActivationFunctionType

NKT = S // P       # 16 key tiles per sequence
KQ = QR // P       # 12
KKV = KVR // P     # 4
KX = D // P        # 16
MT = TLOC // P     # 4 token tiles per core
NBLK = B * (S // 512)  # 8 attention blocks of 512 q tokens
AV_LAG = 3         # PV matmuls trail the score matmuls by this many tiles


def build(has_bias: bool, phases: int = 3):
    nc = bacc.Bacc("TRN2", target_bir_lowering=False, debug=False,
                   num_devices=NCORES, enable_asserts=False)

    def din(name, shape, dt=BF):
        return nc.dram_tensor(name, shape, dt, kind="ExternalInput").ap()

    xt = din("xt", [D, TLOC])
    wq_down = din("wq_down", [D, QR])
    wkv_down = din("wkv_down", [D, KVR])
    gq_up = din("gq_up", [QR, NQ])
    gq_rope = din("gq_rope", [QR, NR])
    gk_up = din("gk_up", [KVR, NQ])
    gk_rope = din("gk_rope", [KVR, NR])
    gv_up = din("gv_up", [KVR, NQ])
    wout = din("wout", [NQ, D])
    cos_t = din("cos_t", [NR, T])
    sin_t = din("sin_t", [NR, T])  # rows d<32 of each 64-block pre-negated
    if has_bias:
        bq_up = din("bq_up", [1, NQ])
        bq_rope = din("bq_rope", [1, NR])
        bk_up = din("bk_up", [1, NQ])
        bk_rope = din("bk_rope", [1, NR])
        bv_up = din("bv_up", [1, NQ])
    out_p = nc.dram_tensor("out_p", [T, D], BF, kind="ExternalOutput").ap()

    agi_kv = nc.dram_tensor("agi_kv", [KVR, TLOC], BF).ap()
    ago_kv = nc.dram_tensor("ago_kv", [NCORES * KVR, TLOC], BF,
                            addr_space="Shared").ap()
    agi_qh = [nc.dram_tensor(f"agi_q{i}", [QR, TLOC // 2], BF).ap()
              for i in range(2)]
    ago_qh = [nc.dram_tensor(f"ago_q{i}", [NCORES * QR, TLOC // 2], BF,
                             addr_space="Shared").ap() for i in range(2)]

    groups = [list(range(NCORES))]

    with tile.TileContext(nc) as tc, ExitStack() as stk:
        # ---------------- constants ----------------
        const = stk.enter_context(tc.tile_pool(name="const", bufs=1))
        ident = const.tile([P, P], BF)
        make_identity(nc, ident)
        ones_tok = const.tile([1, TLOC], BF)
        nc.vector.memset(ones_tok, 1.0)
        ones_row = const.tile([1, P], BF)
        nc.vector.memset(ones_row, 1.0)
        ones_col = const.tile([P, 1], BF)
        nc.vector.memset(ones_col, 1.0)
        eps_t = const.tile([P, 1], F32)
        nc.vector.memset(eps_t, LN_EPS)

        # shared PSUM pool: A,B,C are 2-bank [128,1024] f32 tiles, D is a
        # 1-bank [128,512] double-buffered tile -> 8 banks total.
        psum = stk.enter_context(tc.tile_pool(name="psum", bufs=1, space="PSUM"))

        def big(tag):
            return psum.tile([P, 1024], F32, tag=tag, name="ps" + tag, bufs=1)

        # persistent weight pool; loads are emitted inside phase 1, gated on
        # the kv down-proj weights so they don't steal DMA bandwidth from the
        # startup-critical x / wkv transfers.
        wu = stk.enter_context(tc.tile_pool(name="wu", bufs=1))

        def load_w(dram, rows, cols):
            if rows < P:
                t = wu.tile([rows, 1, cols], BF, name="w_" + dram.tensor.name)
                nc.scalar.dma_start(t[:, 0, :], dram[:, :])
                return t
            t = wu.tile([P, rows // P, cols], BF, name="w_" + dram.tensor.name)
            nc.scalar.dma_start(t[:], dram.rearrange("(k p) n -> p k n", p=P))
            return t

        # ------------- phase 1: down-proj + LN + transpose, kv first -------------
        with (
            tc.tile_pool(name="p1x", bufs=1) as xpool,
            tc.tile_pool(name="p1w", bufs=2) as wpool,
            tc.tile_pool(name="p1c", bufs=1) as cpool,
            tc.tile_pool(name="p1z", bufs=1) as zpool,
            tc.tile_pool(name="p1s", bufs=2) as spool,
        ):
            x_sl = [xpool.tile([P, 4, TLOC], BF, tag=f"x{g}", name=f"x{g}")
                    for g in range(4)]
            xr = xt.rearrange("(k p) m -> p k m", p=P)

            def chunk_w(wdram, col0, also_x=False):
                # weights (and optionally x) in 4 k-slabs so the first
                # matmuls start after ~1MB of DMA, not 4MB
                wr = wdram[:, col0:col0 + 512].rearrange("(k p) n -> p k n", p=P)
                wsl = []
                for g in range(4):
                    w = wpool.tile([P, 4, 512], BF, tag=f"w{g}", name=f"w{g}")
                    nc.sync.dma_start(w[:], wr[:, 4 * g:4 * g + 4, :])
                    if also_x:
                        nc.sync.dma_start(x_sl[g][:], xr[:, 4 * g:4 * g + 4, :])
                    wsl.append(w)
                return wsl

            z_kv = zpool.tile([P, KKV, TLOC], BF, name="z_kv")
            z_q = zpool.tile([P, KQ, TLOC], BF, name="z_q")
            cq_bf = cpool.tile([P, MT, QR], BF, name="cq_bf")
            ssum_kv = cpool.tile([P, MT], F32, name="ssum_kv")
            ssq_kv = cpool.tile([P, MT], F32, name="ssq_kv")
            ssum_q = cpool.tile([P, MT, 3], F32, name="ssum_q")
            ssq_q = cpool.tile([P, MT, 3], F32, name="ssq_q")

            def down_mms(wsl, slots, post=None):
                for k in range(KX):
                    for m in range(MT):
                        nc.tensor.matmul(
                            slots[m], x_sl[k // 4][:, k % 4, m * P:(m + 1) * P],
                            wsl[k // 4][:, k % 4, :],
                            start=(k == 0), stop=(k == KX - 1))
                    if post and k in post:
                        post[k]()

            def slots_ab():
                a, b = big("A"), big("B")
                return [a[:, 0:512], a[:, 512:1024], b[:, 0:512], b[:, 512:1024]]

            def slots_cd():
                c = big("C")
                d0 = psum.tile([P, 512], F32, tag="D", name="psD", bufs=2)
                d1 = psum.tile([P, 512], F32, tag="D", name="psD", bufs=2)
                return [c[:, 0:512], c[:, 512:1024], d0[:], d1[:]]

            def stats(slots, sum_ap, sq_ap, copy_to=None):
                for m in range(MT):
                    nc.vector.reduce_sum(sum_ap(m), slots[m], axis=AX.X)
                    scr = spool.tile([P, 512], BF, tag="scr", name="scr")
                    nc.scalar.activation(scr[:], slots[m], ACT.Square,
                                         accum_out=sq_ap(m))
                    if copy_to is not None:
                        nc.vector.tensor_copy(copy_to(m), slots[m])

            def ln_inv(ssum_ap, ssq_ap, rank, tagsuf):
                mu = spool.tile([P, 1], F32, tag="mu" + tagsuf, name="mu")
                nc.vector.tensor_scalar_mul(mu, ssum_ap, 1.0 / rank)
                musq = spool.tile([P, 1], F32, tag="ms" + tagsuf, name="musq")
                nc.vector.tensor_mul(musq, mu, mu)
                var = spool.tile([P, 1], F32, tag="va" + tagsuf, name="var")
                nc.vector.tensor_scalar_mul(var, ssq_ap, 1.0 / rank)
                nc.vector.tensor_sub(var, var, musq)
                sd = spool.tile([P, 1], F32, tag="sd" + tagsuf, name="sd")
                nc.scalar.activation(sd, var, ACT.Sqrt, bias=eps_t[:])
                inv = spool.tile([P, 1], F32, tag="iv" + tagsuf, name="inv")
                nc.vector.reciprocal(inv, sd)
                return mu, inv

            def transpose_group(cn_ap, zdst, f0, m, tagc):
                # 4 PE transposes into one psum group + a single batched copy
                tpsg = psum.tile([P, 4, P], BF, tag=tagc, name="tpsg", bufs=1)
                for f in range(4):
                    nc.tensor.transpose(tpsg[:, f, :],
                                        cn_ap[:, (f0 + f) * P:(f0 + f + 1) * P],
                                        ident)
                nc.vector.tensor_copy(zdst[:, f0:f0 + 4, m * P:(m + 1) * P],
                                      tpsg[:])

            # --- kv chunk (psum A,B); x DMAs interleaved with kv w slabs ---
            kv_slots = slots_ab()
            wkv_sl = chunk_w(wkv_down, 0, also_x=True)
            down_mms(wkv_sl, kv_slots)
            stats(kv_slots, lambda m: ssum_kv[:, m:m + 1],
                  lambda m: ssq_kv[:, m:m + 1])

            # kv LN math (DVE/ACT); overlaps q chunk 0's matmuls below
            cn_kv = []
            for m in range(MT):
                mu, inv = ln_inv(ssum_kv[:, m:m + 1], ssq_kv[:, m:m + 1],
                                 KVR, "kv")
                cnm = spool.tile([P, KVR], BF, tag="cnkv", name="cn_kv")
                nc.vector.tensor_scalar(cnm[:], kv_slots[m], scalar1=mu,
                                        scalar2=inv, op0=OP.subtract,
                                        op1=OP.mult)
                cn_kv.append(cnm)

            # --- q chunk 0 (psum C,D), kv transposes interleaved mid-loop ---
            def kv_tr(ms):
                def fn():
                    for m in ms:
                        transpose_group(cn_kv[m], z_kv, 0, m,
                                        "A" if m % 2 == 0 else "B")
                return fn
            q0_slots = slots_cd()
            wq0_sl = chunk_w(wq_down, 0)
            down_mms(wq0_sl, q0_slots, post={9: kv_tr((0, 1)),
                                             12: kv_tr((2, 3))})
            nc.sync.dma_start(
                agi_kv.rearrange("(k p) m -> p k m", p=P), z_kv[:])
            nc.gpsimd.collective_compute(
                "AllGather", OP.bypass, ins=[agi_kv[:]], outs=[ago_kv[:]],
                replica_groups=groups)
            stats(q0_slots, lambda m: ssum_q[:, m, 0:1],
                  lambda m: ssq_q[:, m, 0:1],
                  copy_to=lambda m: cq_bf[:, m, 0:512])

            # --- q chunk 1 (A,B) ---
            q1_slots = slots_ab()
            wq1_sl = chunk_w(wq_down, 512)
            down_mms(wq1_sl, q1_slots)
            stats(q1_slots, lambda m: ssum_q[:, m, 1:2],
                  lambda m: ssq_q[:, m, 1:2],
                  copy_to=lambda m: cq_bf[:, m, 512:1024])

            # --- q chunk 2 (C,D), m-major so each token-tile's LayerNorm +
            # transposes run under the next tile's matmuls ---
            q2_slots = slots_cd()
            wq2_sl = chunk_w(wq_down, 1024)

            # persistent weight / rope-table preloads on the scalar queue,
            # gated behind the last down-proj weight slab so they never
            # steal DMA bandwidth from the startup-critical path; ordered
            # by first use (kv up-proj path first).
            gate = spool.tile([1, 1], BF, tag="gate", name="gate")
            nc.scalar.copy(gate[:], wq2_sl[3][0:1, 0, 0:1])
            gku_t = load_w(gk_up, KVR, NQ)
            gkr_t = load_w(gk_rope, KVR, NR)
            gvu_t = load_w(gv_up, KVR, NQ)
            cos_sb = wu.tile([NR, T], BF, name="cos_sb")
            nc.scalar.dma_start(cos_sb[:], cos_t[:, :])
            sin_sb = wu.tile([NR, T], BF, name="sin_sb")
            nc.scalar.dma_start(sin_sb[:], sin_t[:, :])
            gqu_t = load_w(gq_up, QR, NQ)
            gqr_t = load_w(gq_rope, QR, NR)
            wout_t = load_w(wout, NQ, D)
            bias_w = {}
            if has_bias:
                bias_w = dict(bqu=load_w(bq_up, 1, NQ),
                              bqr=load_w(bq_rope, 1, NR),
                              bku=load_w(bk_up, 1, NQ),
                              bkr=load_w(bk_rope, 1, NR),
                              bvu=load_w(bv_up, 1, NQ))

            prev_tr = None
            for m in range(MT):
                for k in range(KX):
                    nc.tensor.matmul(
                        q2_slots[m], x_sl[k // 4][:, k % 4, m * P:(m + 1) * P],
                        wq2_sl[k // 4][:, k % 4, :],
                        start=(k == 0), stop=(k == KX - 1))
                if prev_tr is not None:
                    prev_tr()
                # stats for this tile, then the full-rank LN + normalize
                nc.vector.reduce_sum(ssum_q[:, m, 2:3], q2_slots[m], axis=AX.X)
                scr = spool.tile([P, 512], BF, tag="scr", name="scr")
                nc.scalar.activation(scr[:], q2_slots[m], ACT.Square,
                                     accum_out=ssq_q[:, m, 2:3])
                nc.vector.tensor_copy(cq_bf[:, m, 1024:1536], q2_slots[m])
                st = spool.tile([P, 1], F32, tag="stq", name="st")
                nc.vector.tensor_add(st, ssum_q[:, m, 0:1], ssum_q[:, m, 1:2])
                nc.vector.tensor_add(st, st, ssum_q[:, m, 2:3])
                sq = spool.tile([P, 1], F32, tag="sqq", name="sq")
                nc.vector.tensor_add(sq, ssq_q[:, m, 0:1], ssq_q[:, m, 1:2])
                nc.vector.tensor_add(sq, sq, ssq_q[:, m, 2:3])
                mu, inv = ln_inv(st[:], sq[:], QR, "q")
                cnq = spool.tile([P, QR], BF, tag="cnq", name="cn_q")
                nc.vector.tensor_scalar(cnq[:], cq_bf[:, m, :], scalar1=mu,
                                        scalar2=inv, op0=OP.subtract,
                                        op1=OP.mult)

                def make_tr(cn_ap, mm):
                    def fn():
                        for g in range(3):
                            transpose_group(cn_ap, z_q, 4 * g, mm,
                                            "A" if (mm * 3 + g) % 2 == 0
                                            else "B")
                    return fn
                prev_tr = make_tr(cnq, m)
            prev_tr()

            for i in range(2):
                nc.sync.dma_start(
                    agi_qh[i].rearrange("(k p) m -> p k m", p=P),
                    z_q[:, :, i * 256:(i + 1) * 256])
                nc.gpsimd.collective_compute(
                    "AllGather", OP.bypass, ins=[agi_qh[i][:]],
                    outs=[ago_qh[i][:]], replica_groups=groups)

        if phases < 2:
            out0 = const.tile([P, 512], BF)
            nc.vector.memset(out0, 0.0)
            nc.sync.dma_start(out_p[0:P, 0:512], out0[:])

        # ---------------- phase 2: up-projections + rope ----------------
        qkv = stk.enter_context(tc.tile_pool(name="qkv", bufs=1))
        qc_sb = [qkv.tile([P, T], BF, tag=f"qc{m}", name=f"qc{m}")
                 for m in range(HLOC)]
        kc_sb = [qkv.tile([P, T], BF, tag=f"kc{m}", name=f"kc{m}")
                 for m in range(HLOC)]
        qr_sb = qkv.tile([NR, T], BF, tag="qr", name="qr")
        kr_sb = qkv.tile([NR, T], BF, tag="kr", name="kr")
        v_sb = qkv.tile([P, T // P, NQ], BF, tag="v", name="v")

        rot = {"i": 0}

        def next_half():
            i = rot["i"] % 8
            rot["i"] += 1
            if i >= 6:
                return psum.tile([P, 512], F32, tag="D", name="psD2",
                                 bufs=2)[:]
            t = big("ABC"[i // 2])
            return t[:, (i % 2) * 512:(i % 2) * 512 + 512]

        def proj(zt, nk, wt, mcol0, btile, w=TLOC):
            ps = next_half()[:, 0:w]
            for k in range(nk):
                nc.tensor.matmul(
                    ps, wt[:, k, mcol0:mcol0 + P], zt[:, k, :],
                    start=(k == 0), stop=(k == nk - 1 and btile is None))
            if btile is not None:
                nc.tensor.matmul(ps, btile[:, 0, mcol0:mcol0 + P],
                                 ones_tok[:, 0:w], start=False, stop=True)
            return ps

        with (
            tc.tile_pool(name="p2z", bufs=1) as zpool2,
            tc.tile_pool(name="p2zq", bufs=3) as zqpool,
            tc.tile_pool(name="p2t", bufs=3) as rtmp,
        ):
            def rope_pair(zt, nk, wr, br, dst, csl, w=TLOC):
                # rotate-half computed with partition-shifted multiplies
                # against the sign-baked sin table (no second projection)
                psa = proj(zt, nk, wr, 0, br, w)
                t1 = rtmp.tile([NR, TLOC], BF, tag="t1", name="t1")
                nc.vector.tensor_mul(t1[:, 0:w], psa, cos_sb[:, csl])
                t2 = rtmp.tile([NR, TLOC], BF, tag="t2", name="t2")
                for h in range(HLOC):
                    b = h * RD
                    nc.vector.tensor_mul(t2[b:b + 32, 0:w],
                                         psa[b + 32:b + 64, :],
                                         sin_sb[b:b + 32, csl])
                    nc.vector.tensor_mul(t2[b + 32:b + 64, 0:w],
                                         psa[b:b + 32, :],
                                         sin_sb[b + 32:b + 64, csl])
                nc.vector.tensor_add(dst[:, csl], t1[:, 0:w], t2[:, 0:w])

            # kv path; gathered latents staged in 1MB chunk-pair groups so
            # the first projections start right after the AllGather lands
            zkv_g = []
            for grp in range(4):
                t = zpool2.tile([P, 8, TLOC], BF, tag=f"zkv{grp}",
                                name=f"zkv{grp}")
                nc.sync.dma_start(
                    t[:], ago_kv[grp * 2 * KVR:(grp + 1) * 2 * KVR, :]
                    .rearrange("(k p) m -> p k m", p=P))
                zkv_g.append(t)

            for c in range(NCORES):
                csl = slice(c * TLOC, (c + 1) * TLOC)
                zkv = zkv_g[c // 2][:, (c % 2) * KKV:(c % 2 + 1) * KKV, :]
                for m in range(HLOC):
                    ps = proj(zkv, KKV, gku_t, m * P,
                              bias_w.get("bku"))
                    nc.scalar.copy(kc_sb[m][:, csl], ps)
                rope_pair(zkv, KKV, gkr_t, bias_w.get("bkr"), kr_sb, csl)
                for mp in range(2):  # v: two psum halves, 2 token tiles each
                    ph = next_half()
                    for j in range(2):
                        mt = 2 * mp + j
                        for k in range(KKV):
                            nc.tensor.matmul(
                                ph[:, j * 256:j * 256 + 256],
                                zkv[:, k, mt * P:(mt + 1) * P], gvu_t[:, k, :],
                                start=(k == 0),
                                stop=(k == KKV - 1 and not has_bias))
                        if has_bias:
                            nc.tensor.matmul(ph[:, j * 256:j * 256 + 256],
                                             ones_row[:, :P],
                                             bias_w["bvu"][:, 0, :],
                                             start=False, stop=True)
                    ti = c * MT + 2 * mp
                    nc.scalar.copy(v_sb[:, ti:ti + 2, :], ph)

            # q path: token half A for all chunks (rides under half-B's
            # AllGather), then half B
            for half in range(2):
                for c in range(NCORES):
                    base = c * TLOC + half * 256
                    csl = slice(base, base + 256)
                    zq = zqpool.tile([P, KQ, 256], BF, tag="zq", name="zq")
                    nc.sync.dma_start(
                        zq[:], ago_qh[half][c * QR:(c + 1) * QR, :]
                        .rearrange("(k p) m -> p k m", p=P))
                    for m in range(HLOC):
                        ps = proj(zq, KQ, gqu_t, m * P, bias_w.get("bqu"), 256)
                        nc.scalar.copy(qc_sb[m][:, csl], ps)
                    rope_pair(zq, KQ, gqr_t, bias_w.get("bqr"),
                              qr_sb, csl, 256)

        # ------------- phase 3: pipelined attention + interleaved out-proj -------------
        apool = stk.enter_context(tc.tile_pool(name="p3s", bufs=1))
        ppool = stk.enter_context(tc.tile_pool(name="p3p", bufs=1))
        opool = stk.enter_context(tc.tile_pool(name="p3o", bufs=2))

        acc_t = [apool.tile([P, 1024], BF, tag=f"acc{i}", name=f"acc{i}")
                 for i in range(2)]
        oT_sb = apool.tile([P, 1024], BF, tag="oT", name="oT")
        o_raw = apool.tile([P, 1024], BF, tag="oraw", name="o_raw")
        denf = apool.tile([1, 1024], F32, tag="denf", name="denf")
        rdf_f = apool.tile([1, 1024], F32, tag="rdff", name="rdf_f")
        rsc_t = apool.tile([1, 1024], F32, tag="rsc", name="rsc")
        rd_bf = apool.tile([1, 1024], BF, tag="rdbf", name="rd_bf")
        rb_bf = apool.tile([P, 1024], BF, tag="rbbf", name="rb_bf")

        def make_tail(st):
            """Return slot->emitters finishing block `st` (softmax denom,
            normalize, out-proj, DMA).  Emitted while the NEXT block's score
            stream keeps the PE busy; the denominator reduce/broadcast ride
            on cheap [1,512]-out PE matmuls so no engine ever waits long."""
            if st is None:
                return {}
            tok0 = st["tok0"]
            steps = {}

            def free_o():  # move raw PV sums out of psum so AV(next,0) can start
                nc.vector.tensor_copy(o_raw[:], st["o_ps"][:])
            steps[0] = [free_o]

            def den_mm():  # per-query column sums of exp, both heads
                for h in range(HLOC):
                    dps = psum.tile([1, 512], F32, tag="D", name="dps", bufs=2)
                    nc.tensor.matmul(dps, ones_col[:],
                                     st["acc"][:, h * 512:(h + 1) * 512],
                                     start=True, stop=True)
                    nc.vector.tensor_copy(denf[:, h * 512:(h + 1) * 512], dps)
            steps[1] = [den_mm]

            def recip():
                nc.vector.reciprocal_approx_accurate(rdf_f[:], denf[:], rsc_t[:])
                nc.vector.tensor_copy(rd_bf[:], rdf_f[:])
            steps[2] = [recip]

            def bc():  # broadcast 1/den across the 128 head-dim partitions
                for h in range(HLOC):
                    bps = psum.tile([P, 512], F32, tag="D", name="bps", bufs=2)
                    nc.tensor.matmul(bps, ones_row[:],
                                     rd_bf[:, h * 512:(h + 1) * 512],
                                     start=True, stop=True)
                    nc.vector.tensor_copy(rb_bf[:, h * 512:(h + 1) * 512], bps)
            steps[3] = [bc]

            def norm():
                nc.vector.tensor_mul(oT_sb[:], o_raw[:], rb_bf[:])
            steps[4] = [norm]

            state = {"ot": None}

            def op_group(g):
                def fn():
                    mtl, nch = g // 4, g % 4
                    if nch == 0:
                        state["ot"] = opool.tile([P, D], BF, tag="ot",
                                                 name="ot")
                    po = psum.tile([P, 512], F32, tag="D", name="po", bufs=2)
                    for h in range(HLOC):
                        nc.tensor.matmul(
                            po,
                            oT_sb[:, h * 512 + mtl * P:h * 512 + (mtl + 1) * P],
                            wout_t[:, h, nch * 512:(nch + 1) * 512],
                            start=(h == 0), stop=(h == HLOC - 1))
                    nc.vector.tensor_copy(
                        state["ot"][:, nch * 512:(nch + 1) * 512], po)
                    if nch == 3:
                        r0 = tok0 + mtl * P
                        nc.sync.dma_start(out_p[r0:r0 + P, :], state["ot"][:])
                return fn

            # 16 out-proj groups spread over slots 5..15 and the block end
            done = 0
            for slot in range(5, 16):
                want = (slot - 4) * 16 // 12
                lst = steps.setdefault(slot, [])
                while done < want:
                    lst.append(op_group(done))
                    done += 1
            lst = steps.setdefault("end", [])
            while done < 16:
                lst.append(op_group(done))
                done += 1
            return steps

        def emit_av(o_ps, s, bi, pts):
            pt = pts[s]
            for h in range(HLOC):
                nc.tensor.matmul(
                    o_ps[:, h * 512:(h + 1) * 512],
                    v_sb[:, bi * NKT + s, h * P:(h + 1) * P],
                    pt[:, h * 512:(h + 1) * 512],
                    start=(s == 0), stop=(s == NKT - 1))

        prev = None
        for blk in range(NBLK):
            bi, qch = blk // (S // 512), blk % (S // 512)
            tok0 = bi * S + qch * 512
            tsl = slice(tok0, tok0 + 512)
            acc = acc_t[blk % 2]
            o_ps = big("C")
            tail = make_tail(prev)
            pts = {}
            for s in range(NKT):
                St = big("AB"[s % 2])
                kt0 = bi * S + s * P
                for h in range(HLOC):
                    nc.tensor.matmul(
                        St[:, h * 512:(h + 1) * 512],
                        kc_sb[h][:, kt0:kt0 + P], qc_sb[h][:, tsl],
                        start=True, stop=False)
                for h in range(HLOC):
                    nc.tensor.matmul(
                        St[:, h * 512:(h + 1) * 512],
                        kr_sb[h * RD:(h + 1) * RD, kt0:kt0 + P],
                        qr_sb[h * RD:(h + 1) * RD, tsl],
                        start=False, stop=True)
                pt = ppool.tile([P, 1024], BF, tag=f"pt{s % 4}", name="pt")
                nc.scalar.activation(pt[:], St[:], ACT.Exp)
                if s == 0:
                    nc.vector.tensor_copy(acc[:], pt[:])
                else:
                    with nc.allow_low_precision(
                            reason="softmax denom accum ok in bf16"):
                        nc.vector.tensor_add(acc[:], acc[:], pt[:])
                pts[s] = pt
                for fn in tail.get(s, []):
                    fn()
                if s >= AV_LAG:
                    emit_av(o_ps, s - AV_LAG, bi, pts)
            for s2 in range(NKT - AV_LAG, NKT):
                emit_av(o_ps, s2, bi, pts)
            for fn in tail.get("end", []):
                fn()
            prev = {"tok0": tok0, "acc": acc, "o_ps": o_ps}

        # tail of the final block (PE idles briefly on the denom chain)
        tail = make_tail(prev)
        for slot in list(range(0, 16)) + ["end"]:
            for fn in tail.get(slot, []):
                fn()

    nc.compile()
    return nc


_BUILD_CACHE = {}


def _get_nc(has_bias: bool):
    if has_bias not in _BUILD_CACHE:
        _BUILD_CACHE[has_bias] = build(has_bias)
    return _BUILD_CACHE[has_bias]


def _bf(a):
    return np.ascontiguousarray(a).astype(ml_dtypes.bfloat16)


def _prep_in_maps(x, Wq_down, q_gamma, q_beta, Wq_up, Wq_rope,
                  Wkv_down, kv_gamma, kv_beta, Wk_up, Wv_up, Wk_rope, Wout):
    x = np.asarray(x, dtype=np.float32)
    xT = np.ascontiguousarray(x.reshape(T, D).T)  # [D, T]

    # rope tables, feature-major, duplicated for the 2 local heads; the
    # first half of each 64-row block carries -sin so the kernel's
    # partition-shifted rotate-half multiplies need no sign op
    inv_freq = 1.0 / (10000.0 ** (np.arange(0, RD, 2, dtype=np.float32) / RD))
    pos = (np.arange(T) % S).astype(np.float32)
    freqs = pos[:, None] * inv_freq[None, :]          # [T, 32]
    emb = np.concatenate([freqs, freqs], axis=1)      # [T, 64]
    cosT = np.ascontiguousarray(np.cos(emb).T)        # [64, T]
    sinT = np.ascontiguousarray(np.sin(emb).T)
    sinT[:RD // 2, :] *= -1.0
    cos2 = np.concatenate([cosT, cosT], axis=0)       # [128, T]
    sin2 = np.concatenate([sinT, sinT], axis=0)

    q_gamma = np.asarray(q_gamma, np.float32)
    q_beta = np.asarray(q_beta, np.float32)
    kv_gamma = np.asarray(kv_gamma, np.float32)
    kv_beta = np.asarray(kv_beta, np.float32)
    has_bias = bool(np.any(q_beta) or np.any(kv_beta))

    Wq_up_h = np.asarray(Wq_up, np.float32).reshape(QR, H, HD)
    Wq_rope_h = np.asarray(Wq_rope, np.float32).reshape(QR, H, RD)
    Wk_up_h = np.asarray(Wk_up, np.float32).reshape(KVR, H, HD)
    Wk_rope_h = np.asarray(Wk_rope, np.float32).reshape(KVR, H, RD)
    Wv_up_h = np.asarray(Wv_up, np.float32).reshape(KVR, H, HD)
    Wout_h = np.asarray(Wout, np.float32).reshape(H, HD, D)

    in_maps = []
    for c in range(NCORES):
        hs = slice(HLOC * c, HLOC * (c + 1))
        wq_up_s = Wq_up_h[:, hs].reshape(QR, NQ)
        wq_rope_s = Wq_rope_h[:, hs].reshape(QR, NR)
        wk_up_s = Wk_up_h[:, hs].reshape(KVR, NQ)
        wk_rope_s = Wk_rope_h[:, hs].reshape(KVR, NR)
        wv_up_s = Wv_up_h[:, hs].reshape(KVR, NQ)
        wout_s = Wout_h[hs].reshape(NQ, D)

        gq_up = q_gamma[:, None] * wq_up_s * SCALE
        gq_rope = q_gamma[:, None] * wq_rope_s * SCALE
        gk_up = kv_gamma[:, None] * wk_up_s
        gk_rope = kv_gamma[:, None] * wk_rope_s
        gv_up = kv_gamma[:, None] * wv_up_s

        m = {
            "xt": _bf(xT[:, c * TLOC:(c + 1) * TLOC]),
            "wq_down": _bf(Wq_down),
            "wkv_down": _bf(Wkv_down),
            "gq_up": _bf(gq_up),
            "gq_rope": _bf(gq_rope),
            "gk_up": _bf(gk_up),
            "gk_rope": _bf(gk_rope),
            "gv_up": _bf(gv_up),
            "wout": _bf(wout_s),
            "cos_t": _bf(cos2),
            "sin_t": _bf(sin2),
        }
        if has_bias:
            m["bq_up"] = _bf((q_beta @ wq_up_s * SCALE)[None, :])
            m["bq_rope"] = _bf((q_beta @ wq_rope_s * SCALE)[None, :])
            m["bk_up"] = _bf((kv_beta @ wk_up_s)[None, :])
            m["bk_rope"] = _bf((kv_beta @ wk_rope_s)[None, :])
            m["bv_up"] = _bf((kv_beta @ wv_up_s)[None, :])
        in_maps.append(m)
    return in_maps, has_bias


def kernel(**inputs):
    in_maps, has_bias = _prep_in_maps(**inputs)
    nc = _get_nc(has_bias)
    res = run_bass_kernel_spmd(nc, in_maps, list(range(NCORES)))
    out = res.results[0]["out_p"].astype(np.float32)
    for c in range(1, NCORES):
        out = out + res.results[c]["out_p"].astype(np.float32)
    return out.reshape(B, S, D)
